# revision 1
# baseline (speedup 1.0000x reference)
"""GCN encoder (3x GCNConv sharing one normalized adjacency) on 8 TRN2 NeuronCores.

Strategy:
  - Fold the symmetric GCN norm  norm(r,c) = dis[r]*dis[c]  into per-node
    scales: pre-scale rows by dis (host for x, epilogue for h), post-scale
    aggregates by dis[c]. Per-edge messages then need no per-edge weights.
  - Shard destination nodes across the 8 cores (6272 nodes/core after
    padding N=50000 -> 50176). Edges live on the core that owns their
    destination (edge-cut partitioning per the sharding hint).
  - Per conv: gather source rows with dma_gather (fp16 rows, 256B), build
    {0,1} one-hot matrices on the vector engine (is_equal vs an iota), and
    scatter-add via TensorE matmuls accumulating in PSUM per 128-dst tile.
  - Node features are republished between convs with AllGather collectives.
  - mu and logstd share one pass: Wc = [W_mu | W_logstd] (both 64 wide).
"""

import numpy as np

N = 50000
E = 800000
IN = 128
HID = 128
OUT = 64
NCORES = 8
SH = 6272                 # nodes per core (padded)
NPAD = SH * NCORES        # 50176
NT = SH // 128            # 49 dst tiles per core
LO = 32768                # rows in the "lo" gather table (int16 limit)
HIR = NPAD - LO           # rows in the "hi" gather table
TB = 6                    # dst tiles per gather batch
OHB = 8                   # one-hot chunks generated per DVE op

TRACE = False             # test.py sets this for profiling runs
LAST_RESULTS = None       # test.py reads exec_time_ns from here
DEBUG_STAGE = 0           # 4 = stop after conv1, out_ml rows = hc tiles (f32)

_CACHE = {}


def _preprocess(edge_index):
    src = np.asarray(edge_index[0]).astype(np.int64)
    dst = np.asarray(edge_index[1]).astype(np.int64)
    loop = np.arange(N, dtype=np.int64)
    src_all = np.concatenate([src, loop])
    dst_all = np.concatenate([dst, loop])

    deg = np.bincount(dst_all, minlength=N).astype(np.float32)
    dis = (1.0 / np.sqrt(deg)).astype(np.float32)  # deg >= 1 (self loops)

    per_core = []
    cnts = np.zeros((NCORES, NT, 2), np.int64)
    for c in range(NCORES):
        m = (dst_all // SH) == c
        es = src_all[m]
        ed = dst_all[m] - c * SH
        t = ed >> 7
        dl = ed & 127
        g = (es >= LO).astype(np.int64)
        order = np.lexsort((g, t))
        es, t, dl, g = es[order], t[order], dl[order], g[order]
        key = t * 2 + g
        bc = np.bincount(key, minlength=NT * 2)
        cnts[c] = bc.reshape(NT, 2)
        per_core.append((es, t, dl, g, key))

    C = (cnts.max(axis=0) + 127) // 128        # [NT, 2] chunks per (tile, grp)
    KL = int(C[:, 0].sum())                    # total lo chunks
    KH = int(C[:, 1].sum())                    # total hi chunks
    KT = KL + KH

    lo_off = np.concatenate([[0], np.cumsum(C[:, 0])[:-1]])   # chunk offset in lo stream
    hi_off = np.concatenate([[0], np.cumsum(C[:, 1])[:-1]])
    kk_off = np.concatenate([[0], np.cumsum(C.sum(axis=1))[:-1]])  # global chunk index

    core_data = []
    for c in range(NCORES):
        es, t, dl, g, key = per_core[c]
        # rank of each message within its (tile, grp) block
        blk_start = np.concatenate([[0], np.cumsum(cnts[c].reshape(-1))[:-1]])
        rank = np.arange(len(es)) - blk_start[key]
        # position in the per-group padded stream
        stream_chunk_off = np.where(g == 0, lo_off[t], hi_off[t])
        pos = stream_chunk_off * 128 + rank
        slo = np.zeros(KL * 128, np.int16)
        shi = np.zeros(KH * 128, np.int16)
        slo[pos[g == 0]] = es[g == 0].astype(np.int16)
        shi[pos[g == 1]] = (es[g == 1] - LO).astype(np.int16)
        # destT: global chunk order is per tile [lo chunks..., hi chunks...]
        kk = np.where(g == 0, kk_off[t], kk_off[t] + C[t, 0]) + rank // 128
        dest = np.full(KT * 128, 255.0, np.float16)
        dest[kk * 128 + rank % 128] = dl.astype(np.float16)
        idx_lo = np.tile(slo.reshape(-1, 16).T, (8, 1))   # [128, KL*8]
        idx_hi = np.tile(shi.reshape(-1, 16).T, (8, 1))   # [128, KH*8]
        destT = np.ascontiguousarray(dest.reshape(KT, 128).T)  # [128, KT]
        core_data.append((idx_lo, idx_hi, destT))

    # gather batches: [t0, t1) tile ranges
    batches = []
    t0 = 0
    while t0 < NT:
        t1 = min(t0 + TB, NT)
        batches.append((t0, t1))
        t0 = t1
    meta = dict(C=C, KL=KL, KH=KH, KT=KT,
                lo_off=lo_off, hi_off=hi_off, kk_off=kk_off, batches=batches)
    return dis, core_data, meta


def _build_nc(meta):
    import concourse.bass as bass
    import concourse.bacc as bacc
    import concourse.mybir as mybir
    import concourse.tile as tile
    from concourse import library_config

    C = meta["C"]
    KL, KH, KT = meta["KL"], meta["KH"], meta["KT"]
    lo_off, hi_off, kk_off = meta["lo_off"], meta["hi_off"], meta["kk_off"]
    batches = meta["batches"]

    f16 = mybir.dt.float16
    f32 = mybir.dt.float32
    i16 = mybir.dt.int16
    eq = mybir.AluOpType.is_equal
    mult = mybir.AluOpType.mult
    add = mybir.AluOpType.add
    amax = mybir.AluOpType.max

    nc = bacc.Bacc("TRN2", target_bir_lowering=False, debug=False,
                   enable_asserts=True, num_devices=NCORES)

    xTs = nc.dram_tensor("xTs", [128, SH], f16, kind="ExternalInput")
    W1d = nc.dram_tensor("W1d", [128, 128], f16, kind="ExternalInput")
    Wcd = nc.dram_tensor("Wcd", [128, 128], f16, kind="ExternalInput")
    b1rd = nc.dram_tensor("b1rd", [128, 128], f32, kind="ExternalInput")
    bcrd = nc.dram_tensor("bcrd", [128, 128], f32, kind="ExternalInput")
    disT32d = nc.dram_tensor("disT32d", [128, NT], f32, kind="ExternalInput")
    disT16d = nc.dram_tensor("disT16d", [128, NT], f16, kind="ExternalInput")
    iotad = nc.dram_tensor("iotad", [128, OHB * 128], f16, kind="ExternalInput")
    identd = nc.dram_tensor("identd", [128, 128], f16, kind="ExternalInput")
    idxlod = nc.dram_tensor("idxlod", [128, KL * 8], i16, kind="ExternalInput")
    idxhid = nc.dram_tensor("idxhid", [128, KH * 8], i16, kind="ExternalInput")
    destTd = nc.dram_tensor("destTd", [128, KT], f16, kind="ExternalInput")
    out_ml = nc.dram_tensor("out_ml", [SH, 128], f32, kind="ExternalOutput")

    with tile.TileContext(nc) as tc:
        with (
            tc.tile_pool(name="consts", bufs=1) as cpool,
            tc.tile_pool(name="xin", bufs=3) as xpool,
            tc.tile_pool(name="work", bufs=3) as wpool,
            tc.tile_pool(name="oh", bufs=3) as ohpool,
            tc.tile_pool(name="glo", bufs=2) as gpool_lo,
            tc.tile_pool(name="ghi", bufs=2) as gpool_hi,
            tc.tile_pool(name="psA", bufs=2, space="PSUM") as psA,
            tc.tile_pool(name="psB", bufs=2, space="PSUM") as psB,
            tc.tile_pool(name="psT", bufs=2, space="PSUM") as psT,
            tc.tile_pool(name="psH", bufs=2, space="PSUM") as psH,
            tc.tile_pool(name="dram", bufs=1, space="DRAM") as dpool,
        ):
            nc.gpsimd.load_library(library_config.mlp)

            W1sb = cpool.tile([128, 128], f16, tag="W1sb")
            Wcsb = cpool.tile([128, 128], f16, tag="Wcsb")
            b1sb = cpool.tile([128, 128], f32, tag="b1sb")
            bcsb = cpool.tile([128, 128], f32, tag="bcsb")
            dis32sb = cpool.tile([128, NT], f32, tag="dis32sb")
            dis16sb = cpool.tile([128, NT], f16, tag="dis16sb")
            iotasb = cpool.tile([128, OHB * 128], f16, tag="iotasb")
            identsb = cpool.tile([128, 128], f16, tag="identsb")
            idxlosb = cpool.tile([128, KL * 8], i16, tag="idxlosb")
            idxhisb = cpool.tile([128, KH * 8], i16, tag="idxhisb")
            destTsb = cpool.tile([128, KT], f16, tag="destTsb")

            nc.sync.dma_start(W1sb[:], W1d.ap())
            nc.sync.dma_start(Wcsb[:], Wcd.ap())
            nc.sync.dma_start(b1sb[:], b1rd.ap())
            nc.sync.dma_start(bcsb[:], bcrd.ap())
            nc.sync.dma_start(dis32sb[:], disT32d.ap())
            nc.sync.dma_start(dis16sb[:], disT16d.ap())
            nc.sync.dma_start(iotasb[:], iotad.ap())
            nc.sync.dma_start(identsb[:], identd.ap())
            nc.sync.dma_start(idxlosb[:], idxlod.ap())
            nc.sync.dma_start(idxhisb[:], idxhid.ap())
            nc.sync.dma_start(destTsb[:], destTd.ap())

            h0s = dpool.tile([SH, 128], f16, tag="h0s")
            h0f = dpool.tile([NPAD, 128], f16, tag="h0f")
            hcs = dpool.tile([SH, 128], f16, tag="hcs")
            hcf = dpool.tile([NPAD, 128], f16, tag="hcf")

            # ---- Phase A: h0' shard = (x*dis)@W1, rows of my shard ----
            for t in range(NT):
                xt = xpool.tile([128, 128], f16, tag="xt")
                nc.sync.dma_start(xt[:], xTs.ap()[:, t * 128:(t + 1) * 128])
                ps = psA.tile([128, 128], f32, tag="psA")
                nc.tensor.matmul(ps[:], xt[:], W1sb[:], start=True, stop=True)
                ht = xpool.tile([128, 128], f16, tag="ht")
                nc.scalar.copy(ht[:], ps[:])
                nc.sync.dma_start(h0s[t * 128:(t + 1) * 128, :], ht[:])

            nc.gpsimd.collective_compute(
                "AllGather", mybir.AluOpType.bypass,
                replica_groups=[list(range(NCORES))],
                ins=[h0s.opt()], outs=[h0f.opt()],
            )

            def conv_pass(table, is_conv1):
                for (t0, t1) in batches:
                    cl = int(C[t0:t1, 0].sum())
                    ch = int(C[t0:t1, 1].sum())
                    glo = ghi = None
                    if cl:
                        glo = gpool_lo.tile([128, cl, 128], f16, tag="glo")
                        nc.gpsimd.dma_gather(
                            glo[:], table[0:LO, :],
                            idxlosb[:, int(lo_off[t0]) * 8:(int(lo_off[t0]) + cl) * 8],
                            num_idxs=cl * 128, num_idxs_reg=cl * 128,
                            elem_size=128, single_packet=False,
                        )
                    if ch:
                        ghi = gpool_hi.tile([128, ch, 128], f16, tag="ghi")
                        nc.gpsimd.dma_gather(
                            ghi[:], table[LO:NPAD, :],
                            idxhisb[:, int(hi_off[t0]) * 8:(int(hi_off[t0]) + ch) * 8],
                            num_idxs=ch * 128, num_idxs_reg=ch * 128,
                            elem_size=128, single_packet=False,
                        )
                    for t in range(t0, t1):
                        nch = int(C[t, 0] + C[t, 1])
                        kk0 = int(kk_off[t])
                        # one-hot matrices for all chunks of this tile
                        ohs = []
                        j = 0
                        while j < nch:
                            nb = min(OHB, nch - j)
                            oh = ohpool.tile([128, nb, 128], f16, tag="oh")
                            nc.vector.tensor_tensor(
                                oh[:],
                                iotasb[:, 0:nb * 128].rearrange(
                                    "p (c e) -> p c e", e=128),
                                destTsb[:, kk0 + j:kk0 + j + nb].broadcast_to(
                                    [128, nb, 128]),
                                eq,
                            )
                            ohs.append((j, nb, oh))
                            j += nb

                        def oh_at(k):
                            for (jj, nb, oh) in ohs:
                                if jj <= k < jj + nb:
                                    return oh[:, k - jj, :]
                            raise AssertionError

                        ps = psB.tile([128, 128], f32, tag="psB")
                        k = 0
                        for j2 in range(int(C[t, 0])):
                            src = glo[:, int(lo_off[t] - lo_off[t0]) + j2, :]
                            nc.tensor.matmul(ps[:], oh_at(k), src,
                                             start=(k == 0), stop=(k == nch - 1),
                                             skip_group_check=True)
                            k += 1
                        for j2 in range(int(C[t, 1])):
                            src = ghi[:, int(hi_off[t] - hi_off[t0]) + j2, :]
                            nc.tensor.matmul(ps[:], oh_at(k), src,
                                             start=(k == 0), stop=(k == nch - 1),
                                             skip_group_check=True)
                            k += 1

                        if is_conv1:
                            # h = relu(dis*agg + b1); hs = h*dis
                            hti = wpool.tile([128, 128], f16, tag="hti")
                            nc.vector.scalar_tensor_tensor(
                                hti[:], ps[:], dis32sb[:, t:t + 1], b1sb[:],
                                mult, add)
                            hst = wpool.tile([128, 128], f16, tag="hst")
                            nc.vector.tensor_scalar(
                                hst[:], hti[:], 0.0, dis32sb[:, t:t + 1],
                                amax, mult)
                            # hsT = transpose(hs); hc tile = hsT.T @ Wc
                            pst = psT.tile([128, 128], f16, tag="psT")
                            nc.tensor.transpose(pst[:], hst[:], identsb[:])
                            hsT = wpool.tile([128, 128], f16, tag="hsT")
                            nc.scalar.copy(hsT[:], pst[:])
                            psh = psH.tile([128, 128], f32, tag="psH")
                            nc.tensor.matmul(psh[:], hsT[:], Wcsb[:],
                                             start=True, stop=True,
                                             skip_group_check=True)
                            hct = wpool.tile([128, 128], f16, tag="hct")
                            nc.scalar.copy(hct[:], psh[:])
                            nc.sync.dma_start(hcs[t * 128:(t + 1) * 128, :],
                                              hct[:])
                            if DEBUG_STAGE == 4:
                                dbg = wpool.tile([128, 128], f32, tag="dbg")
                                nc.vector.tensor_copy(dbg[:], psh[:])
                                nc.sync.dma_start(
                                    out_ml.ap()[t * 128:(t + 1) * 128, :],
                                    dbg[:])
                        else:
                            ot = wpool.tile([128, 128], f32, tag="ot")
                            if DEBUG_STAGE == 8:
                                nc.vector.tensor_copy(ot[:], ps[:])
                            else:
                                nc.vector.scalar_tensor_tensor(
                                    ot[:], ps[:], dis32sb[:, t:t + 1], bcsb[:],
                                    mult, add)
                            nc.sync.dma_start(out_ml.ap()[t * 128:(t + 1) * 128, :],
                                              ot[:])

            conv_pass(h0f, True)

            if DEBUG_STAGE != 4:
                nc.gpsimd.collective_compute(
                    "AllGather", mybir.AluOpType.bypass,
                    replica_groups=[list(range(NCORES))],
                    ins=[hcs.opt()], outs=[hcf.opt()],
                )

                if DEBUG_STAGE == 7:
                    for t in range(NT):
                        tt = wpool.tile([128, 128], f16, tag="dbg7a")
                        nc.sync.dma_start(tt[:], hcf[t * 128:(t + 1) * 128, :])
                        of = wpool.tile([128, 128], f32, tag="dbg7b")
                        nc.scalar.copy(of[:], tt[:])
                        nc.sync.dma_start(
                            out_ml.ap()[t * 128:(t + 1) * 128, :], of[:])
                else:
                    conv_pass(hcf, False)

    nc.compile()
    return nc


def kernel(x, edge_index, W1, b1, W_mu, b_mu, W_logstd, b_logstd):
    global LAST_RESULTS
    from concourse.bass_utils import run_bass_kernel_spmd

    x = np.asarray(x, dtype=np.float32)
    W1 = np.asarray(W1, dtype=np.float32)
    b1 = np.asarray(b1, dtype=np.float32)
    W_mu = np.asarray(W_mu, dtype=np.float32)
    b_mu = np.asarray(b_mu, dtype=np.float32)
    W_logstd = np.asarray(W_logstd, dtype=np.float32)
    b_logstd = np.asarray(b_logstd, dtype=np.float32)

    key = np.asarray(edge_index).tobytes()[:64] + np.asarray(edge_index).tobytes()[-64:]
    cached = _CACHE.get("k")
    if cached is not None and cached[0] == key:
        _, dis, core_data, meta, nc = cached
    else:
        dis, core_data, meta = _preprocess(edge_index)
        nc = _build_nc(meta)
        _CACHE["k"] = (key, dis, core_data, meta, nc)

    # host-side tensors
    x2T = np.zeros((IN, NPAD), np.float16)
    x2T[:, :N] = (x * dis[:, None]).T.astype(np.float16)
    W1h = W1.astype(np.float16)
    Wch = np.concatenate([W_mu, W_logstd], axis=1).astype(np.float16)
    b1r = np.tile(b1[None, :], (128, 1)).astype(np.float32)
    bcr = np.tile(np.concatenate([b_mu, b_logstd])[None, :], (128, 1)).astype(np.float32)
    disP = np.zeros(NPAD, np.float32)
    disP[:N] = dis
    iota = np.tile(np.arange(128, dtype=np.float16)[None, :], (128, OHB))
    ident = np.eye(128, dtype=np.float16)

    in_maps = []
    for c in range(NCORES):
        idx_lo, idx_hi, destT = core_data[c]
        disSh = disP[c * SH:(c + 1) * SH].reshape(NT, 128).T  # [128, NT]
        in_maps.append({
            "xTs": np.ascontiguousarray(x2T[:, c * SH:(c + 1) * SH]),
            "W1d": W1h, "Wcd": Wch, "b1rd": b1r, "bcrd": bcr,
            "disT32d": np.ascontiguousarray(disSh.astype(np.float32)),
            "disT16d": np.ascontiguousarray(disSh.astype(np.float16)),
            "iotad": np.ascontiguousarray(iota),
            "identd": ident,
            "idxlod": idx_lo, "idxhid": idx_hi, "destTd": destT,
        })

    res = run_bass_kernel_spmd(nc, in_maps, core_ids=list(range(NCORES)),
                               trace=TRACE)
    LAST_RESULTS = res
    full = np.concatenate([res.results[c]["out_ml"] for c in range(NCORES)],
                          axis=0)[:N]
    mu = np.ascontiguousarray(full[:, :OUT])
    logstd = np.ascontiguousarray(full[:, OUT:])
    return (mu, logstd)



# revision 6
# speedup vs baseline: 2.1557x; 2.1557x over previous
"""GCN encoder (3x GCNConv sharing one normalized adjacency) on 8 TRN2 NeuronCores.

v2 design (vs baseline):
  - Fold sym-norm into per-node scales (as before): pre-scale rows by dis,
    post-scale aggregates by dis[dst].
  - Conv1 gathers directly from a replicated row-major (x*dis) table in HBM
    and aggregates raw input rows TRANSPOSED (psum[feat,dst] += chunk.T@OH);
    W1 is applied once per dst tile afterwards. This removes the dense
    pre-GEMM phase and the first AllGather entirely.
  - dma_gather descriptor generation is the machine bottleneck (~7.7ns/desc
    on one Q7 core pair). Gathers are spread over 4 SWDGE queues so 4 core
    pairs generate descriptors concurrently.
  - One-hot scatter matrices are precomputed on the host in fp8e4 (exact
    {0,1}) and streamed from HBM - no DVE is_equal generation at all.
  - Self-loop messages are removed from the gather streams; each dst tile
    adds its local rows via one identity matmul.
  - mu and logstd share one pass: Wc = [W_mu | W_logstd].
"""

import numpy as np
import ml_dtypes

N = 50000
E = 800000
IN = 128
HID = 128
OUT = 64
NCORES = 8
SH = 6272                 # nodes per core (padded)
NPAD = SH * NCORES        # 50176
NT = SH // 128            # 49 dst tiles per core
LO = 32768                # rows in the "lo" gather table (int16 limit)
TB = 4                    # dst tiles per gather batch
NQ = 4                    # SWDGE queues

TRACE = False             # test.py sets this for profiling runs
LAST_RESULTS = None       # test.py reads exec_time_ns from here

_CACHE = {}


def _preprocess(edge_index):
    src = np.asarray(edge_index[0]).astype(np.int64)
    dst = np.asarray(edge_index[1]).astype(np.int64)
    loop = np.arange(N, dtype=np.int64)
    dst_all = np.concatenate([dst, loop])

    deg = np.bincount(dst_all, minlength=N).astype(np.float32)
    dis = (1.0 / np.sqrt(deg)).astype(np.float32)  # deg >= 1 (self loops)

    # per-core edge streams (no self loops; dst-sharded)
    per_core = []
    cnts = np.zeros((NCORES, NT, 2), np.int64)
    for c in range(NCORES):
        m = (dst // SH) == c
        es = src[m]
        ed = dst[m] - c * SH
        t = ed >> 7
        dl = ed & 127
        g = (es >= LO).astype(np.int64)
        order = np.lexsort((g, t))
        es, t, dl, g = es[order], t[order], dl[order], g[order]
        key = t * 2 + g
        bc = np.bincount(key, minlength=NT * 2)
        cnts[c] = bc.reshape(NT, 2)
        per_core.append((es, t, dl, g, key))

    C = (cnts.max(axis=0) + 127) // 128        # [NT, 2] chunks per (tile, grp)
    KL = int(C[:, 0].sum())                    # total lo chunks
    KH = int(C[:, 1].sum())                    # total hi chunks
    KT = KL + KH

    lo_off = np.concatenate([[0], np.cumsum(C[:, 0])[:-1]])   # chunk offset in lo stream
    hi_off = np.concatenate([[0], np.cumsum(C[:, 1])[:-1]])
    kk_off = np.concatenate([[0], np.cumsum(C.sum(axis=1))[:-1]])  # global chunk index

    core_data = []
    for c in range(NCORES):
        es, t, dl, g, key = per_core[c]
        blk_start = np.concatenate([[0], np.cumsum(cnts[c].reshape(-1))[:-1]])
        rank = np.arange(len(es)) - blk_start[key]
        stream_chunk_off = np.where(g == 0, lo_off[t], hi_off[t])
        pos = stream_chunk_off * 128 + rank
        slo = np.zeros(KL * 128, np.int16)
        shi = np.zeros(KH * 128, np.int16)
        slo[pos[g == 0]] = es[g == 0].astype(np.int16)
        shi[pos[g == 1]] = (es[g == 1] - LO).astype(np.int16)
        # one-hot fp8 matrices: chunk kk (tile-major: lo chunks then hi):
        # OH[lane, kk*128 + dstlane] = 1 for each message
        kk = np.where(g == 0, kk_off[t], kk_off[t] + C[t, 0]) + rank // 128
        oh = np.zeros((128, KT * 128), np.uint8)
        oh[rank % 128, kk * 128 + dl] = 0x38    # 1.0 in fp8 e4m3
        idx_lo = np.tile(slo.reshape(-1, 16).T, (8, 1))   # [128, KL*8]
        idx_hi = np.tile(shi.reshape(-1, 16).T, (8, 1))   # [128, KH*8]
        core_data.append((idx_lo, idx_hi, oh.view(ml_dtypes.float8_e4m3)))

    batches = []
    t0 = 0
    while t0 < NT:
        t1 = min(t0 + TB, NT)
        batches.append((t0, t1))
        t0 = t1
    meta = dict(C=C, KL=KL, KH=KH, KT=KT,
                lo_off=lo_off, hi_off=hi_off, kk_off=kk_off, batches=batches)
    return dis, core_data, meta


def _build_nc(meta):
    import concourse.bass as bass
    import concourse.bacc as bacc
    import concourse.mybir as mybir
    import concourse.tile as tile
    from concourse import library_config

    C = meta["C"]
    KL, KH, KT = meta["KL"], meta["KH"], meta["KT"]
    lo_off, hi_off, kk_off = meta["lo_off"], meta["hi_off"], meta["kk_off"]
    batches = meta["batches"]

    f16 = mybir.dt.float16
    f32 = mybir.dt.float32
    f8 = mybir.dt.float8e4
    i16 = mybir.dt.int16
    mult = mybir.AluOpType.mult
    add = mybir.AluOpType.add
    amax = mybir.AluOpType.max

    nc = bacc.Bacc("TRN2", target_bir_lowering=False, debug=False,
                   enable_asserts=True, num_devices=NCORES,
                   num_swdge_queues=NQ)

    x2Rd = nc.dram_tensor("x2Rd", [NPAD, 128], f16, kind="ExternalInput")
    xlocd = nc.dram_tensor("xlocd", [SH, 128], f16, kind="ExternalInput")
    W1d = nc.dram_tensor("W1d", [128, 128], f16, kind="ExternalInput")
    Wcd = nc.dram_tensor("Wcd", [128, 128], f16, kind="ExternalInput")
    b1rd = nc.dram_tensor("b1rd", [128, 128], f32, kind="ExternalInput")
    bcrd = nc.dram_tensor("bcrd", [128, 128], f32, kind="ExternalInput")
    disT32d = nc.dram_tensor("disT32d", [128, NT], f32, kind="ExternalInput")
    identd = nc.dram_tensor("identd", [128, 128], f8, kind="ExternalInput")
    idxlod = nc.dram_tensor("idxlod", [128, KL * 8], i16, kind="ExternalInput")
    idxhid = nc.dram_tensor("idxhid", [128, KH * 8], i16, kind="ExternalInput")
    ohd = nc.dram_tensor("ohd", [128, KT * 128], f8, kind="ExternalInput")
    out_ml = nc.dram_tensor("out_ml", [SH, 128], f32, kind="ExternalOutput")

    with tile.TileContext(nc) as tc:
        with (
            tc.tile_pool(name="consts", bufs=1) as cpool,
            tc.tile_pool(name="loc", bufs=3) as lpool,
            tc.tile_pool(name="work", bufs=3) as wpool,
            tc.tile_pool(name="oh", bufs=3) as ohpool,
            tc.tile_pool(name="glo", bufs=3) as gpool_lo,
            tc.tile_pool(name="ghi", bufs=3) as gpool_hi,
            tc.tile_pool(name="psA", bufs=4, space="PSUM") as psA,
            tc.tile_pool(name="psH", bufs=2, space="PSUM") as psH,
            tc.tile_pool(name="dram", bufs=1, space="DRAM") as dpool,
        ):
            nc.gpsimd.load_library(library_config.mlp)

            W1sb = cpool.tile([128, 128], f16, tag="W1sb")
            Wcsb = cpool.tile([128, 128], f16, tag="Wcsb")
            b1sb = cpool.tile([128, 128], f32, tag="b1sb")
            bcsb = cpool.tile([128, 128], f32, tag="bcsb")
            dis32sb = cpool.tile([128, NT], f32, tag="dis32sb")
            identsb = cpool.tile([128, 128], f8, tag="identsb")
            idxlosb = cpool.tile([128, KL * 8], i16, tag="idxlosb")
            idxhisb = cpool.tile([128, KH * 8], i16, tag="idxhisb")

            nc.sync.dma_start(W1sb[:], W1d.ap())
            nc.sync.dma_start(Wcsb[:], Wcd.ap())
            nc.sync.dma_start(b1sb[:], b1rd.ap())
            nc.sync.dma_start(bcsb[:], bcrd.ap())
            nc.sync.dma_start(dis32sb[:], disT32d.ap())
            nc.sync.dma_start(identsb[:], identd.ap())
            nc.sync.dma_start(idxlosb[:], idxlod.ap())
            nc.sync.dma_start(idxhisb[:], idxhid.ap())

            hcs = dpool.tile([SH, 128], f16, tag="hcs")
            hcf = dpool.tile([NPAD, 128], f16, tag="hcf", addr_space="Shared")

            def conv_pass(table, loc_src, is_conv1):
                for bi, (t0, t1) in enumerate(batches):
                    cl = int(C[t0:t1, 0].sum())
                    ch = int(C[t0:t1, 1].sum())
                    nbk = int(kk_off[t1 - 1] + C[t1 - 1].sum() - kk_off[t0])
                    # one-hot slab for this batch (contiguous in ohd)
                    ohsb = ohpool.tile([128, nbk * 128], f8, tag="ohsb")
                    nc.sync.dma_start(
                        ohsb[:],
                        ohd.ap()[:, int(kk_off[t0]) * 128:
                                 (int(kk_off[t0]) + nbk) * 128])
                    glo = ghi = None
                    if cl:
                        glo = gpool_lo.tile([128, cl, 128], f16, tag="glo")
                        nc.gpsimd.dma_gather(
                            glo[:], table[0:LO, :],
                            idxlosb[:, int(lo_off[t0]) * 8:(int(lo_off[t0]) + cl) * 8],
                            num_idxs=cl * 128, num_idxs_reg=cl * 128,
                            elem_size=128, single_packet=False,
                            queue_num=bi % NQ,
                        )
                    if ch:
                        ghi = gpool_hi.tile([128, ch, 128], f16, tag="ghi")
                        nc.gpsimd.dma_gather(
                            ghi[:], table[LO:NPAD, :],
                            idxhisb[:, int(hi_off[t0]) * 8:(int(hi_off[t0]) + ch) * 8],
                            num_idxs=ch * 128, num_idxs_reg=ch * 128,
                            elem_size=128, single_packet=False,
                            queue_num=(bi + 2) % NQ,
                        )
                    for t in range(t0, t1):
                        nch = int(C[t, 0] + C[t, 1])
                        kk0 = int(kk_off[t] - kk_off[t0])
                        # local tile rows for the self-loop contribution
                        lt = lpool.tile([128, 128], f16, tag="lt")
                        nc.sync.dma_start(
                            lt[:], loc_src[t * 128:(t + 1) * 128, :])
                        ps = psA.tile([128, 128], f32, tag="psA")
                        # psum[feat, dst]: self loops via identity, then chunks
                        nc.tensor.matmul(ps[:], lt[:], identsb[:],
                                         start=True, stop=(nch == 0),
                                         skip_group_check=True)
                        k = 0
                        for j2 in range(int(C[t, 0])):
                            src = glo[:, int(lo_off[t] - lo_off[t0]) + j2, :]
                            nc.tensor.matmul(
                                ps[:], src,
                                ohsb[:, (kk0 + k) * 128:(kk0 + k + 1) * 128],
                                start=False, stop=(k == nch - 1),
                                skip_group_check=True)
                            k += 1
                        for j2 in range(int(C[t, 1])):
                            src = ghi[:, int(hi_off[t] - hi_off[t0]) + j2, :]
                            nc.tensor.matmul(
                                ps[:], src,
                                ohsb[:, (kk0 + k) * 128:(kk0 + k + 1) * 128],
                                start=False, stop=(k == nch - 1),
                                skip_group_check=True)
                            k += 1

                        # aggT [feat, dst] -> f16, then @ W gives [dst, wout]
                        aggT = wpool.tile([128, 128], f16, tag="aggT")
                        nc.scalar.copy(aggT[:], ps[:])
                        psh = psH.tile([128, 128], f32, tag="psH")
                        nc.tensor.matmul(psh[:], aggT[:],
                                         W1sb[:] if is_conv1 else Wcsb[:],
                                         start=True, stop=True,
                                         skip_group_check=True)
                        if is_conv1:
                            # h = relu(dis*psh + b1); hs = dis*h
                            hti = wpool.tile([128, 128], f32, tag="hti")
                            nc.vector.scalar_tensor_tensor(
                                hti[:], psh[:], dis32sb[:, t:t + 1], b1sb[:],
                                mult, add)
                            hct = wpool.tile([128, 128], f16, tag="hct")
                            nc.scalar.activation(
                                hct[:], hti[:], mybir.ActivationFunctionType.Relu,
                                scale=dis32sb[:, t:t + 1])
                            nc.sync.dma_start(hcs[t * 128:(t + 1) * 128, :],
                                              hct[:])
                        else:
                            ot = wpool.tile([128, 128], f32, tag="ot")
                            nc.vector.scalar_tensor_tensor(
                                ot[:], psh[:], dis32sb[:, t:t + 1], bcsb[:],
                                mult, add)
                            nc.sync.dma_start(out_ml.ap()[t * 128:(t + 1) * 128, :],
                                              ot[:])

            conv_pass(x2Rd, xlocd, True)

            nc.gpsimd.collective_compute(
                "AllGather", mybir.AluOpType.bypass,
                replica_groups=[list(range(NCORES))],
                ins=[hcs.opt()], outs=[hcf.opt()],
            )

            conv_pass(hcf, hcs, False)

    nc.compile()
    return nc


def kernel(x, edge_index, W1, b1, W_mu, b_mu, W_logstd, b_logstd):
    global LAST_RESULTS
    from concourse.bass_utils import run_bass_kernel_spmd

    x = np.asarray(x, dtype=np.float32)
    W1 = np.asarray(W1, dtype=np.float32)
    b1 = np.asarray(b1, dtype=np.float32)
    W_mu = np.asarray(W_mu, dtype=np.float32)
    b_mu = np.asarray(b_mu, dtype=np.float32)
    W_logstd = np.asarray(W_logstd, dtype=np.float32)
    b_logstd = np.asarray(b_logstd, dtype=np.float32)

    key = np.asarray(edge_index).tobytes()[:64] + np.asarray(edge_index).tobytes()[-64:]
    cached = _CACHE.get("k")
    if cached is not None and cached[0] == key:
        _, dis, core_data, meta, nc = cached
    else:
        dis, core_data, meta = _preprocess(edge_index)
        nc = _build_nc(meta)
        _CACHE["k"] = (key, dis, core_data, meta, nc)

    # host-side tensors
    x2R = np.zeros((NPAD, 128), np.float16)
    x2R[:N] = (x * dis[:, None]).astype(np.float16)
    W1h = W1.astype(np.float16)
    Wch = np.concatenate([W_mu, W_logstd], axis=1).astype(np.float16)
    b1r = np.tile(b1[None, :], (128, 1)).astype(np.float32)
    bcr = np.tile(np.concatenate([b_mu, b_logstd])[None, :], (128, 1)).astype(np.float32)
    disP = np.zeros(NPAD, np.float32)
    disP[:N] = dis
    ident = np.zeros((128, 128), np.uint8)
    ident[np.arange(128), np.arange(128)] = 0x38
    ident = ident.view(ml_dtypes.float8_e4m3)

    in_maps = []
    for c in range(NCORES):
        idx_lo, idx_hi, oh = core_data[c]
        disSh = disP[c * SH:(c + 1) * SH].reshape(NT, 128).T  # [128, NT]
        in_maps.append({
            "x2Rd": x2R,
            "xlocd": np.ascontiguousarray(x2R[c * SH:(c + 1) * SH]),
            "W1d": W1h, "Wcd": Wch, "b1rd": b1r, "bcrd": bcr,
            "disT32d": np.ascontiguousarray(disSh.astype(np.float32)),
            "identd": ident,
            "idxlod": idx_lo, "idxhid": idx_hi, "ohd": oh,
        })

    res = run_bass_kernel_spmd(nc, in_maps, core_ids=list(range(NCORES)),
                               trace=TRACE)
    LAST_RESULTS = res
    full = np.concatenate([res.results[c]["out_ml"] for c in range(NCORES)],
                          axis=0)[:N]
    mu = np.ascontiguousarray(full[:, :OUT])
    logstd = np.ascontiguousarray(full[:, OUT:])
    return (mu, logstd)


# revision 8
# speedup vs baseline: 2.6295x; 1.2198x over previous
"""GCN encoder (3x GCNConv sharing one normalized adjacency) on 8 TRN2 NeuronCores.

v3 design:
  - Fold sym-norm into per-node scales: pre-scale rows by dis, post-scale
    aggregates by dis[dst].
  - Conv1 gathers directly from a replicated row-major (x*dis) table in HBM
    and aggregates raw input rows TRANSPOSED (psum[feat,dst] += chunk.T@OH);
    W1 is applied once per dst tile afterwards. No dense pre-GEMM, no first
    AllGather.
  - dma_gather descriptor generation runs on one Q7 core pair per SWDGE
    queue (~7.9ns/desc); gathers rotate over 4 queues so 4 pairs generate
    concurrently. 256B random HBM reads then become the wall (~0.35-0.5
    accesses/ns); deep buffering (TB=2 tile batches, 6 gather bufs) keeps
    the SDMA queues full, and per-block source-sorting improves locality.
  - One-hot scatter matrices precomputed on the host in fp8e4 and streamed
    from HBM (no DVE is_equal).
  - Self loops leave the gather streams; each dst tile adds its local rows
    via one identity matmul.
  - The republish AllGather is split in two chunks (tiles 0-24 / 25-48 of
    each shard) so chunk A overlaps the tail of conv1 and pass-2 gathers on
    table A overlap AllGather B. Pass 2 has its own group split (by chunk
    table), idx streams, and one-hots.
  - mu and logstd share one pass: Wc = [W_mu | W_logstd].
"""

import numpy as np
import ml_dtypes

N = 50000
E = 800000
IN = 128
HID = 128
OUT = 64
NCORES = 8
SH = 6272                 # nodes per core (padded)
NPAD = SH * NCORES        # 50176
NT = SH // 128            # 49 dst tiles per core
LO = 32768                # rows in pass-1 "lo" table (int16 limit)
TSPLIT = 25               # pass-2 chunk A = tiles [0,25), B = [25,49)
RA = TSPLIT * 128         # 3200 rows per shard in chunk A
RB = SH - RA              # 3072 rows per shard in chunk B
NROWA = NCORES * RA       # 25600 (< 32767: int16 ok)
NROWB = NCORES * RB       # 24576
TB = 2                    # dst tiles per gather batch
NQ = 4                    # SWDGE queues

TRACE = False             # test.py sets this for profiling runs
LAST_RESULTS = None       # test.py reads exec_time_ns from here

_CACHE = {}


def _build_streams(es_tab, t, dl, g, ngrp):
    """Build per-core padded gather streams + fp8 one-hots for one pass.

    es_tab: per-message index into its group's table
    t: dst tile; dl: dst lane; g: group id (0..ngrp-1)
    All arrays are lists per core. Returns dict with C [NT,ngrp], offsets,
    per-core idx streams (per group) and OH fp8 arrays.
    """
    cnts = np.zeros((NCORES, NT, ngrp), np.int64)
    ordered = []
    for c in range(NCORES):
        order = np.lexsort((es_tab[c], g[c], t[c]))  # by tile, grp, src (locality)
        e, tt, dd, gg = es_tab[c][order], t[c][order], dl[c][order], g[c][order]
        key = tt * ngrp + gg
        bc = np.bincount(key, minlength=NT * ngrp)
        cnts[c] = bc.reshape(NT, ngrp)
        ordered.append((e, tt, dd, gg, key))

    C = (cnts.max(axis=0) + 127) // 128            # [NT, ngrp]
    K = C.sum(axis=0).astype(np.int64)             # chunks per group stream
    KT = int(C.sum())
    g_off = np.concatenate([np.zeros((1, ngrp), np.int64),
                            np.cumsum(C, axis=0)[:-1]], axis=0)  # [NT, ngrp]
    kk_off = np.concatenate([[0], np.cumsum(C.sum(axis=1))[:-1]])

    per_core = []
    for c in range(NCORES):
        e, tt, dd, gg, key = ordered[c]
        blk_start = np.concatenate([[0], np.cumsum(cnts[c].reshape(-1))[:-1]])
        rank = np.arange(len(e)) - blk_start[key]
        pos = g_off[tt, gg] * 128 + rank           # position in group stream
        streams = []
        for gi in range(ngrp):
            s = np.zeros(int(K[gi]) * 128, np.int16)
            m = gg == gi
            s[pos[m]] = e[m].astype(np.int16)
            streams.append(np.tile(s.reshape(-1, 16).T, (8, 1)))  # [128, K*8]
        # chunk index: tile-major, groups in order within tile
        cum_in_tile = np.concatenate(
            [np.zeros((NT, 1), np.int64), np.cumsum(C, axis=1)[:, :-1]], axis=1)
        kk = kk_off[tt] + cum_in_tile[tt, gg] + rank // 128
        oh = np.zeros((128, KT * 128), np.uint8)
        oh[rank % 128, kk * 128 + dd] = 0x38       # 1.0 in fp8 e4m3
        per_core.append((streams, oh.view(ml_dtypes.float8_e4m3)))

    return dict(C=C, K=K, KT=KT, g_off=g_off, kk_off=kk_off,
                per_core=per_core, ngrp=ngrp)


def _preprocess(edge_index):
    src = np.asarray(edge_index[0]).astype(np.int64)
    dst = np.asarray(edge_index[1]).astype(np.int64)
    loop = np.arange(N, dtype=np.int64)
    dst_all = np.concatenate([dst, loop])

    deg = np.bincount(dst_all, minlength=N).astype(np.float32)
    dis = (1.0 / np.sqrt(deg)).astype(np.float32)

    es_by_core, t_by_core, dl_by_core = [], [], []
    for c in range(NCORES):
        m = (dst // SH) == c
        es = src[m]
        ed = dst[m] - c * SH
        es_by_core.append(es)
        t_by_core.append(ed >> 7)
        dl_by_core.append(ed & 127)

    # pass 1: table = x2R [NPAD,128]; groups lo (idx<LO) / hi
    g1 = [(e >= LO).astype(np.int64) for e in es_by_core]
    e1 = [np.where(e >= LO, e - LO, e) for e in es_by_core]
    p1 = _build_streams(e1, t_by_core, dl_by_core, g1, 2)

    # pass 2: tables hcfA [NROWA,128] / hcfB [NROWB,128]
    # node (c2, r) -> table A pos c2*RA + r if r < RA else B pos c2*RB + (r-RA)
    g2, e2 = [], []
    for c in range(NCORES):
        e = es_by_core[c]
        c2, r = e // SH, e % SH
        inB = (r >= RA).astype(np.int64)
        g2.append(inB)
        e2.append(np.where(inB == 1, c2 * RB + (r - RA), c2 * RA + r))
    p2 = _build_streams(e2, t_by_core, dl_by_core, g2, 2)

    batches = []
    t0 = 0
    while t0 < NT:
        t1 = min(t0 + TB, NT)
        batches.append((t0, t1))
        t0 = t1
    return dis, dict(p1=p1, p2=p2, batches=batches)


def _build_nc(meta):
    import concourse.bass as bass
    import concourse.bacc as bacc
    import concourse.mybir as mybir
    import concourse.tile as tile
    from concourse import library_config

    batches = meta["batches"]
    p1, p2 = meta["p1"], meta["p2"]

    f16 = mybir.dt.float16
    f32 = mybir.dt.float32
    f8 = mybir.dt.float8e4
    i16 = mybir.dt.int16
    mult = mybir.AluOpType.mult
    add = mybir.AluOpType.add

    nc = bacc.Bacc("TRN2", target_bir_lowering=False, debug=False,
                   enable_asserts=True, num_devices=NCORES,
                   num_swdge_queues=NQ)

    x2Rd = nc.dram_tensor("x2Rd", [NPAD, 128], f16, kind="ExternalInput")
    xlocd = nc.dram_tensor("xlocd", [SH, 128], f16, kind="ExternalInput")
    W1d = nc.dram_tensor("W1d", [128, 128], f16, kind="ExternalInput")
    Wcd = nc.dram_tensor("Wcd", [128, 128], f16, kind="ExternalInput")
    b1rd = nc.dram_tensor("b1rd", [128, 128], f32, kind="ExternalInput")
    bcrd = nc.dram_tensor("bcrd", [128, 128], f32, kind="ExternalInput")
    disT32d = nc.dram_tensor("disT32d", [128, NT], f32, kind="ExternalInput")
    identd = nc.dram_tensor("identd", [128, 128], f8, kind="ExternalInput")
    idx1 = [nc.dram_tensor(f"idx1g{g}", [128, int(p1["K"][g]) * 8], i16,
                           kind="ExternalInput") for g in range(2)]
    idx2 = [nc.dram_tensor(f"idx2g{g}", [128, int(p2["K"][g]) * 8], i16,
                           kind="ExternalInput") for g in range(2)]
    oh1d = nc.dram_tensor("oh1d", [128, p1["KT"] * 128], f8, kind="ExternalInput")
    oh2d = nc.dram_tensor("oh2d", [128, p2["KT"] * 128], f8, kind="ExternalInput")
    out_ml = nc.dram_tensor("out_ml", [SH, 128], f32, kind="ExternalOutput")

    with tile.TileContext(nc) as tc:
        with (
            tc.tile_pool(name="consts", bufs=1) as cpool,
            tc.tile_pool(name="loc", bufs=6) as lpool,
            tc.tile_pool(name="work", bufs=4) as wpool,
            tc.tile_pool(name="oh", bufs=4) as ohpool,
            tc.tile_pool(name="g0", bufs=6) as gpool0,
            tc.tile_pool(name="g1", bufs=6) as gpool1,
            tc.tile_pool(name="psA", bufs=4, space="PSUM") as psA,
            tc.tile_pool(name="psH", bufs=2, space="PSUM") as psH,
            tc.tile_pool(name="dram", bufs=1, space="DRAM") as dpool,
        ):
            nc.gpsimd.load_library(library_config.mlp)

            W1sb = cpool.tile([128, 128], f16, tag="W1sb")
            Wcsb = cpool.tile([128, 128], f16, tag="Wcsb")
            b1sb = cpool.tile([128, 128], f32, tag="b1sb")
            bcsb = cpool.tile([128, 128], f32, tag="bcsb")
            dis32sb = cpool.tile([128, NT], f32, tag="dis32sb")
            identsb = cpool.tile([128, 128], f8, tag="identsb")
            idx1sb = [cpool.tile([128, int(p1["K"][g]) * 8], i16,
                                 tag=f"idx1g{g}", name=f"idx1sb{g}")
                      for g in range(2)]
            idx2sb = [cpool.tile([128, int(p2["K"][g]) * 8], i16,
                                 tag=f"idx2g{g}", name=f"idx2sb{g}")
                      for g in range(2)]

            nc.sync.dma_start(W1sb[:], W1d.ap())
            nc.sync.dma_start(Wcsb[:], Wcd.ap())
            nc.sync.dma_start(b1sb[:], b1rd.ap())
            nc.sync.dma_start(bcsb[:], bcrd.ap())
            nc.sync.dma_start(dis32sb[:], disT32d.ap())
            nc.sync.dma_start(identsb[:], identd.ap())
            for g in range(2):
                nc.sync.dma_start(idx1sb[g][:], idx1[g].ap())
                nc.sync.dma_start(idx2sb[g][:], idx2[g].ap())

            hcsA = dpool.tile([RA, 128], f16, tag="hcsA")
            hcsB = dpool.tile([RB, 128], f16, tag="hcsB")
            hcfA = dpool.tile([NROWA, 128], f16, tag="hcfA", addr_space="Shared")
            hcfB = dpool.tile([NROWB, 128], f16, tag="hcfB", addr_space="Shared")

            def conv_pass(pp, tables, idxsb, ohd_t, loc_src, is_conv1):
                C, g_off, kk_off = pp["C"], pp["g_off"], pp["kk_off"]
                for bi, (t0, t1) in enumerate(batches):
                    nbk = int(kk_off[t1 - 1] + C[t1 - 1].sum() - kk_off[t0])
                    ohsb = ohpool.tile([128, nbk * 128], f8, tag="ohsb")
                    nc.sync.dma_start(
                        ohsb[:],
                        ohd_t.ap()[:, int(kk_off[t0]) * 128:
                                   (int(kk_off[t0]) + nbk) * 128])
                    gts = []
                    for g in range(2):
                        cg = int(C[t0:t1, g].sum())
                        if cg == 0:
                            gts.append(None)
                            continue
                        pool = gpool0 if g == 0 else gpool1
                        gt = pool.tile([128, cg, 128], f16, tag=f"gt{g}")
                        o0 = int(g_off[t0, g])
                        nc.gpsimd.dma_gather(
                            gt[:], tables[g],
                            idxsb[g][:, o0 * 8:(o0 + cg) * 8],
                            num_idxs=cg * 128, num_idxs_reg=cg * 128,
                            elem_size=128, single_packet=False,
                            queue_num=(bi + 2 * g) % NQ,
                        )
                        gts.append(gt)
                    for t in range(t0, t1):
                        nch = int(C[t].sum())
                        kk0 = int(kk_off[t] - kk_off[t0])
                        lt = lpool.tile([128, 128], f16, tag="lt")
                        nc.sync.dma_start(
                            lt[:], loc_src(t))
                        ps = psA.tile([128, 128], f32, tag="psA")
                        nc.tensor.matmul(ps[:], lt[:], identsb[:],
                                         start=True, stop=(nch == 0),
                                         skip_group_check=True)
                        k = 0
                        for g in range(2):
                            for j2 in range(int(C[t, g])):
                                src = gts[g][:, int(g_off[t, g] - g_off[t0, g]) + j2, :]
                                nc.tensor.matmul(
                                    ps[:], src,
                                    ohsb[:, (kk0 + k) * 128:(kk0 + k + 1) * 128],
                                    start=False, stop=(k == nch - 1),
                                    skip_group_check=True)
                                k += 1

                        aggT = wpool.tile([128, 128], f16, tag="aggT")
                        nc.scalar.copy(aggT[:], ps[:])
                        psh = psH.tile([128, 128], f32, tag="psH")
                        nc.tensor.matmul(psh[:], aggT[:],
                                         W1sb[:] if is_conv1 else Wcsb[:],
                                         start=True, stop=True,
                                         skip_group_check=True)
                        if is_conv1:
                            # h = relu(dis*psh + b1); hs = dis*h
                            hti = wpool.tile([128, 128], f32, tag="hti")
                            nc.vector.scalar_tensor_tensor(
                                hti[:], psh[:], dis32sb[:, t:t + 1], b1sb[:],
                                mult, add)
                            hct = wpool.tile([128, 128], f16, tag="hct")
                            nc.scalar.activation(
                                hct[:], hti[:], mybir.ActivationFunctionType.Relu,
                                scale=dis32sb[:, t:t + 1])
                            if t < TSPLIT:
                                nc.sync.dma_start(
                                    hcsA[t * 128:(t + 1) * 128, :], hct[:])
                            else:
                                nc.sync.dma_start(
                                    hcsB[(t - TSPLIT) * 128:(t - TSPLIT + 1) * 128, :],
                                    hct[:])
                        else:
                            ot = wpool.tile([128, 128], f32, tag="ot")
                            nc.vector.scalar_tensor_tensor(
                                ot[:], psh[:], dis32sb[:, t:t + 1], bcsb[:],
                                mult, add)
                            nc.sync.dma_start(out_ml.ap()[t * 128:(t + 1) * 128, :],
                                              ot[:])

            def loc1(t):
                return xlocd[t * 128:(t + 1) * 128, :]

            def loc2(t):
                if t < TSPLIT:
                    return hcsA[t * 128:(t + 1) * 128, :]
                return hcsB[(t - TSPLIT) * 128:(t - TSPLIT + 1) * 128, :]

            conv_pass(p1, [x2Rd[0:LO, :], x2Rd[LO:NPAD, :]], idx1sb, oh1d,
                      loc1, True)

            nc.gpsimd.collective_compute(
                "AllGather", mybir.AluOpType.bypass,
                replica_groups=[list(range(NCORES))],
                ins=[hcsA.opt()], outs=[hcfA.opt()],
            )
            nc.gpsimd.collective_compute(
                "AllGather", mybir.AluOpType.bypass,
                replica_groups=[list(range(NCORES))],
                ins=[hcsB.opt()], outs=[hcfB.opt()],
            )

            conv_pass(p2, [hcfA[:], hcfB[:]], idx2sb, oh2d, loc2, False)

    nc.compile()
    return nc


def kernel(x, edge_index, W1, b1, W_mu, b_mu, W_logstd, b_logstd):
    global LAST_RESULTS
    from concourse.bass_utils import run_bass_kernel_spmd

    x = np.asarray(x, dtype=np.float32)
    W1 = np.asarray(W1, dtype=np.float32)
    b1 = np.asarray(b1, dtype=np.float32)
    W_mu = np.asarray(W_mu, dtype=np.float32)
    b_mu = np.asarray(b_mu, dtype=np.float32)
    W_logstd = np.asarray(W_logstd, dtype=np.float32)
    b_logstd = np.asarray(b_logstd, dtype=np.float32)

    key = np.asarray(edge_index).tobytes()[:64] + np.asarray(edge_index).tobytes()[-64:]
    cached = _CACHE.get("k")
    if cached is not None and cached[0] == key:
        _, dis, meta, nc = cached
    else:
        dis, meta = _preprocess(edge_index)
        nc = _build_nc(meta)
        _CACHE["k"] = (key, dis, meta, nc)

    x2R = np.zeros((NPAD, 128), np.float16)
    x2R[:N] = (x * dis[:, None]).astype(np.float16)
    W1h = W1.astype(np.float16)
    Wch = np.concatenate([W_mu, W_logstd], axis=1).astype(np.float16)
    b1r = np.tile(b1[None, :], (128, 1)).astype(np.float32)
    bcr = np.tile(np.concatenate([b_mu, b_logstd])[None, :], (128, 1)).astype(np.float32)
    disP = np.zeros(NPAD, np.float32)
    disP[:N] = dis
    ident = np.zeros((128, 128), np.uint8)
    ident[np.arange(128), np.arange(128)] = 0x38
    ident = ident.view(ml_dtypes.float8_e4m3)

    in_maps = []
    for c in range(NCORES):
        s1, oh1 = meta["p1"]["per_core"][c]
        s2, oh2 = meta["p2"]["per_core"][c]
        disSh = disP[c * SH:(c + 1) * SH].reshape(NT, 128).T  # [128, NT]
        in_maps.append({
            "x2Rd": x2R,
            "xlocd": np.ascontiguousarray(x2R[c * SH:(c + 1) * SH]),
            "W1d": W1h, "Wcd": Wch, "b1rd": b1r, "bcrd": bcr,
            "disT32d": np.ascontiguousarray(disSh.astype(np.float32)),
            "identd": ident,
            "idx1g0": s1[0], "idx1g1": s1[1],
            "idx2g0": s2[0], "idx2g1": s2[1],
            "oh1d": oh1, "oh2d": oh2,
        })

    res = run_bass_kernel_spmd(nc, in_maps, core_ids=list(range(NCORES)),
                               trace=TRACE)
    LAST_RESULTS = res
    full = np.concatenate([res.results[c]["out_ml"] for c in range(NCORES)],
                          axis=0)[:N]
    mu = np.ascontiguousarray(full[:, :OUT])
    logstd = np.ascontiguousarray(full[:, OUT:])
    return (mu, logstd)


# revision 15
# speedup vs baseline: 2.6872x; 1.0219x over previous
"""GCN encoder (3x GCNConv sharing one normalized adjacency) on 8 TRN2 NeuronCores.

v3 design:
  - Fold sym-norm into per-node scales: pre-scale rows by dis, post-scale
    aggregates by dis[dst].
  - Conv1 gathers directly from a replicated row-major (x*dis) table in HBM
    and aggregates raw input rows TRANSPOSED (psum[feat,dst] += chunk.T@OH);
    W1 is applied once per dst tile afterwards. No dense pre-GEMM, no first
    AllGather.
  - dma_gather descriptor generation runs on one Q7 core pair per SWDGE
    queue (~7.9ns/desc); gathers rotate over 4 queues so 4 pairs generate
    concurrently. 256B random HBM reads then become the wall (~0.35-0.5
    accesses/ns); deep buffering (TB=2 tile batches, 6 gather bufs) keeps
    the SDMA queues full, and per-block source-sorting improves locality.
  - One-hot scatter matrices precomputed on the host in fp8e4 and streamed
    from HBM (no DVE is_equal).
  - Self loops leave the gather streams; each dst tile adds its local rows
    via one identity matmul.
  - The republish AllGather is split in two chunks (tiles 0-24 / 25-48 of
    each shard) so chunk A overlaps the tail of conv1 and pass-2 gathers on
    table A overlap AllGather B. Pass 2 has its own group split (by chunk
    table), idx streams, and one-hots.
  - mu and logstd share one pass: Wc = [W_mu | W_logstd].
"""

import numpy as np
import ml_dtypes

N = 50000
E = 800000
IN = 128
HID = 128
OUT = 64
NCORES = 8
SH = 6272                 # nodes per core (padded)
NPAD = SH * NCORES        # 50176
NT = SH // 128            # 49 dst tiles per core
LO = 32768                # rows in pass-1 "lo" table (int16 limit)
TSPLIT = 25               # pass-2 chunk A = tiles [0,25), B = [25,49)
RA = TSPLIT * 128         # 3200 rows per shard in chunk A
RB = SH - RA              # 3072 rows per shard in chunk B
NROWA = NCORES * RA       # 25600 (< 32767: int16 ok)
NROWB = NCORES * RB       # 24576
TB = 2                    # dst tiles per gather batch
NQ = 4                    # SWDGE queues
AGA_AT = 14               # issue AllGather-A after this batch of pass 1

TRACE = False             # test.py sets this for profiling runs
LAST_RESULTS = None       # test.py reads exec_time_ns from here

_CACHE = {}


def _build_streams(es_tab, t, dl, g, ngrp):
    """Build per-core padded gather streams + fp8 one-hots for one pass.

    es_tab: per-message index into its group's table
    t: dst tile; dl: dst lane; g: group id (0..ngrp-1)
    All arrays are lists per core. Returns dict with C [NT,ngrp], offsets,
    per-core idx streams (per group) and OH fp8 arrays.
    """
    cnts = np.zeros((NCORES, NT, ngrp), np.int64)
    ordered = []
    for c in range(NCORES):
        order = np.lexsort((es_tab[c], g[c], t[c]))  # by tile, grp, src (locality)
        e, tt, dd, gg = es_tab[c][order], t[c][order], dl[c][order], g[c][order]
        key = tt * ngrp + gg
        bc = np.bincount(key, minlength=NT * ngrp)
        cnts[c] = bc.reshape(NT, ngrp)
        ordered.append((e, tt, dd, gg, key))

    C = (cnts.max(axis=0) + 127) // 128            # [NT, ngrp]
    K = C.sum(axis=0).astype(np.int64)             # chunks per group stream
    KT = int(C.sum())
    g_off = np.concatenate([np.zeros((1, ngrp), np.int64),
                            np.cumsum(C, axis=0)[:-1]], axis=0)  # [NT, ngrp]
    kk_off = np.concatenate([[0], np.cumsum(C.sum(axis=1))[:-1]])

    per_core = []
    for c in range(NCORES):
        e, tt, dd, gg, key = ordered[c]
        blk_start = np.concatenate([[0], np.cumsum(cnts[c].reshape(-1))[:-1]])
        rank = np.arange(len(e)) - blk_start[key]
        pos = g_off[tt, gg] * 128 + rank           # position in group stream
        streams = []
        for gi in range(ngrp):
            s = np.zeros(int(K[gi]) * 128, np.int16)
            m = gg == gi
            s[pos[m]] = e[m].astype(np.int16)
            streams.append(np.tile(s.reshape(-1, 16).T, (8, 1)))  # [128, K*8]
        # chunk index: tile-major, groups in order within tile
        cum_in_tile = np.concatenate(
            [np.zeros((NT, 1), np.int64), np.cumsum(C, axis=1)[:, :-1]], axis=1)
        kk = kk_off[tt] + cum_in_tile[tt, gg] + rank // 128
        oh = np.zeros((128, KT * 128), np.uint8)
        oh[rank % 128, kk * 128 + dd] = 0x38       # 1.0 in fp8 e4m3
        per_core.append((streams, oh.view(ml_dtypes.float8_e4m3)))

    return dict(C=C, K=K, KT=KT, g_off=g_off, kk_off=kk_off,
                per_core=per_core, ngrp=ngrp)


def _preprocess(edge_index):
    src = np.asarray(edge_index[0]).astype(np.int64)
    dst = np.asarray(edge_index[1]).astype(np.int64)
    loop = np.arange(N, dtype=np.int64)
    dst_all = np.concatenate([dst, loop])

    deg = np.bincount(dst_all, minlength=N).astype(np.float32)
    dis = (1.0 / np.sqrt(deg)).astype(np.float32)

    es_by_core, t_by_core, dl_by_core = [], [], []
    for c in range(NCORES):
        m = (dst // SH) == c
        es = src[m]
        ed = dst[m] - c * SH
        es_by_core.append(es)
        t_by_core.append(ed >> 7)
        dl_by_core.append(ed & 127)

    # pass 1: table = x2R [NPAD,128]; groups lo (idx<LO) / hi
    g1 = [(e >= LO).astype(np.int64) for e in es_by_core]
    e1 = [np.where(e >= LO, e - LO, e) for e in es_by_core]
    p1 = _build_streams(e1, t_by_core, dl_by_core, g1, 2)

    # pass 2: tables hcfA [NROWA,128] / hcfB [NROWB,128]
    # node (c2, r) -> table A pos c2*RA + r if r < RA else B pos c2*RB + (r-RA)
    g2, e2 = [], []
    for c in range(NCORES):
        e = es_by_core[c]
        c2, r = e // SH, e % SH
        inB = (r >= RA).astype(np.int64)
        g2.append(inB)
        e2.append(np.where(inB == 1, c2 * RB + (r - RA), c2 * RA + r))
    p2 = _build_streams(e2, t_by_core, dl_by_core, g2, 2)

    batches = []
    t0 = 0
    while t0 < NT:
        t1 = min(t0 + TB, NT)
        batches.append((t0, t1))
        t0 = t1
    return dis, dict(p1=p1, p2=p2, batches=batches)


def _build_nc(meta):
    import concourse.bass as bass
    import concourse.bacc as bacc
    import concourse.mybir as mybir
    import concourse.tile as tile
    from concourse import library_config

    batches = meta["batches"]
    p1, p2 = meta["p1"], meta["p2"]

    f16 = mybir.dt.float16
    f32 = mybir.dt.float32
    f8 = mybir.dt.float8e4
    i16 = mybir.dt.int16
    mult = mybir.AluOpType.mult
    add = mybir.AluOpType.add

    nc = bacc.Bacc("TRN2", target_bir_lowering=False, debug=False,
                   enable_asserts=True, num_devices=NCORES,
                   num_swdge_queues=NQ)

    x2Rd = nc.dram_tensor("x2Rd", [NPAD, 128], f16, kind="ExternalInput")
    xlocd = nc.dram_tensor("xlocd", [SH, 128], f16, kind="ExternalInput")
    W1d = nc.dram_tensor("W1d", [128, 128], f16, kind="ExternalInput")
    Wcd = nc.dram_tensor("Wcd", [128, 128], f16, kind="ExternalInput")
    b1rd = nc.dram_tensor("b1rd", [128, 128], f32, kind="ExternalInput")
    bcrd = nc.dram_tensor("bcrd", [128, 128], f32, kind="ExternalInput")
    disT32d = nc.dram_tensor("disT32d", [128, NT], f32, kind="ExternalInput")
    identd = nc.dram_tensor("identd", [128, 128], f8, kind="ExternalInput")
    idx1 = [nc.dram_tensor(f"idx1g{g}", [128, int(p1["K"][g]) * 8], i16,
                           kind="ExternalInput") for g in range(2)]
    idx2 = [nc.dram_tensor(f"idx2g{g}", [128, int(p2["K"][g]) * 8], i16,
                           kind="ExternalInput") for g in range(2)]
    oh1d = nc.dram_tensor("oh1d", [128, p1["KT"] * 128], f8, kind="ExternalInput")
    oh2d = nc.dram_tensor("oh2d", [128, p2["KT"] * 128], f8, kind="ExternalInput")
    out_ml = nc.dram_tensor("out_ml", [SH, 128], f32, kind="ExternalOutput")

    with tile.TileContext(nc) as tc:
        with (
            tc.tile_pool(name="consts", bufs=1) as cpool,
            tc.tile_pool(name="work", bufs=4) as wpool,
            tc.tile_pool(name="oh", bufs=4) as ohpool,
            tc.tile_pool(name="g0", bufs=6) as gpool0,
            tc.tile_pool(name="g1", bufs=6) as gpool1,
            tc.tile_pool(name="psA", bufs=4, space="PSUM") as psA,
            tc.tile_pool(name="psH", bufs=2, space="PSUM") as psH,
            tc.tile_pool(name="dram", bufs=1, space="DRAM") as dpool,
        ):
            nc.gpsimd.load_library(library_config.mlp)

            W1sb = cpool.tile([128, 128], f16, tag="W1sb")
            Wcsb = cpool.tile([128, 128], f16, tag="Wcsb")
            b1sb = cpool.tile([128, 128], f32, tag="b1sb")
            bcsb = cpool.tile([128, 128], f32, tag="bcsb")
            dis32sb = cpool.tile([128, NT], f32, tag="dis32sb")
            identsb = cpool.tile([128, 128], f8, tag="identsb")
            idx1sb = [cpool.tile([128, int(p1["K"][g]) * 8], i16,
                                 tag=f"idx1g{g}", name=f"idx1sb{g}")
                      for g in range(2)]
            idx2sb = [cpool.tile([128, int(p2["K"][g]) * 8], i16,
                                 tag=f"idx2g{g}", name=f"idx2sb{g}")
                      for g in range(2)]

            xres = cpool.tile([128, NT * 128], f16, tag="xres")
            hsres = cpool.tile([128, NT * 128], f16, tag="hsres")

            nc.sync.dma_start(W1sb[:], W1d.ap())
            nc.sync.dma_start(Wcsb[:], Wcd.ap())
            nc.sync.dma_start(b1sb[:], b1rd.ap())
            nc.sync.dma_start(bcsb[:], bcrd.ap())
            nc.sync.dma_start(dis32sb[:], disT32d.ap())
            nc.sync.dma_start(identsb[:], identd.ap())
            for g in range(2):
                nc.sync.dma_start(idx1sb[g][:], idx1[g].ap())
                nc.sync.dma_start(idx2sb[g][:], idx2[g].ap())
            for t in range(NT):
                nc.sync.dma_start(xres[:, t * 128:(t + 1) * 128],
                                  xlocd[t * 128:(t + 1) * 128, :])

            hcsA = dpool.tile([RA, 128], f16, tag="hcsA")
            hcsB = dpool.tile([RB, 128], f16, tag="hcsB")
            hcfA = dpool.tile([NROWA, 128], f16, tag="hcfA", addr_space="Shared")
            hcfB = dpool.tile([NROWB, 128], f16, tag="hcfB", addr_space="Shared")

            def conv_pass(pp, tables, idxsb, ohd_t, loc_res, is_conv1,
                          mid_cb=None):
                C, g_off, kk_off = pp["C"], pp["g_off"], pp["kk_off"]
                for bi, (t0, t1) in enumerate(batches):
                    if mid_cb is not None and bi == AGA_AT + 1:
                        mid_cb()
                    nbk = int(kk_off[t1 - 1] + C[t1 - 1].sum() - kk_off[t0])
                    ohsb = ohpool.tile([128, nbk * 128], f8, tag="ohsb")
                    nc.scalar.dma_start(
                        ohsb[:],
                        ohd_t.ap()[:, int(kk_off[t0]) * 128:
                                   (int(kk_off[t0]) + nbk) * 128])
                    gts = []
                    for g in range(2):
                        cg = int(C[t0:t1, g].sum())
                        if cg == 0:
                            gts.append(None)
                            continue
                        pool = gpool0 if g == 0 else gpool1
                        gt = pool.tile([128, cg, 128], f16, tag=f"gt{g}")
                        o0 = int(g_off[t0, g])
                        nc.gpsimd.dma_gather(
                            gt[:], tables[g],
                            idxsb[g][:, o0 * 8:(o0 + cg) * 8],
                            num_idxs=cg * 128, num_idxs_reg=cg * 128,
                            elem_size=128, single_packet=False,
                            queue_num=(bi + 2 * g) % NQ,
                        )
                        gts.append(gt)
                    for t in range(t0, t1):
                        nch = int(C[t].sum())
                        kk0 = int(kk_off[t] - kk_off[t0])
                        ps = psA.tile([128, 128], f32, tag="psA")
                        nc.tensor.matmul(ps[:],
                                         loc_res[:, t * 128:(t + 1) * 128],
                                         identsb[:],
                                         start=True, stop=(nch == 0),
                                         skip_group_check=True)
                        k = 0
                        for g in range(2):
                            for j2 in range(int(C[t, g])):
                                src = gts[g][:, int(g_off[t, g] - g_off[t0, g]) + j2, :]
                                nc.tensor.matmul(
                                    ps[:], src,
                                    ohsb[:, (kk0 + k) * 128:(kk0 + k + 1) * 128],
                                    start=False, stop=(k == nch - 1),
                                    skip_group_check=True)
                                k += 1

                        aggT = wpool.tile([128, 128], f16, tag="aggT")
                        nc.scalar.copy(aggT[:], ps[:])
                        psh = psH.tile([128, 128], f32, tag="psH")
                        nc.tensor.matmul(psh[:], aggT[:],
                                         W1sb[:] if is_conv1 else Wcsb[:],
                                         start=True, stop=True,
                                         skip_group_check=True)
                        if is_conv1:
                            # h = relu(dis*psh + b1); hs = dis*h
                            hti = wpool.tile([128, 128], f32, tag="hti")
                            nc.vector.scalar_tensor_tensor(
                                hti[:], psh[:], dis32sb[:, t:t + 1], b1sb[:],
                                mult, add)
                            hct = hsres[:, t * 128:(t + 1) * 128]
                            nc.scalar.activation(
                                hct, hti[:], mybir.ActivationFunctionType.Relu,
                                scale=dis32sb[:, t:t + 1])
                            if t < TSPLIT:
                                nc.sync.dma_start(
                                    hcsA[t * 128:(t + 1) * 128, :], hct)
                            else:
                                nc.sync.dma_start(
                                    hcsB[(t - TSPLIT) * 128:(t - TSPLIT + 1) * 128, :],
                                    hct)
                        else:
                            ot = wpool.tile([128, 128], f32, tag="ot")
                            nc.vector.scalar_tensor_tensor(
                                ot[:], psh[:], dis32sb[:, t:t + 1], bcsb[:],
                                mult, add)
                            nc.sync.dma_start(out_ml.ap()[t * 128:(t + 1) * 128, :],
                                              ot[:])

            def issue_agA():
                nc.gpsimd.collective_compute(
                    "AllGather", mybir.AluOpType.bypass,
                    replica_groups=[list(range(NCORES))],
                    ins=[hcsA.opt()], outs=[hcfA.opt()],
                )

            conv_pass(p1, [x2Rd[0:LO, :], x2Rd[LO:NPAD, :]], idx1sb, oh1d,
                      xres, True, mid_cb=issue_agA)

            nc.gpsimd.collective_compute(
                "AllGather", mybir.AluOpType.bypass,
                replica_groups=[list(range(NCORES))],
                ins=[hcsB.opt()], outs=[hcfB.opt()],
            )

            conv_pass(p2, [hcfA[:], hcfB[:]], idx2sb, oh2d, hsres, False)

    nc.compile()
    return nc


def kernel(x, edge_index, W1, b1, W_mu, b_mu, W_logstd, b_logstd):
    global LAST_RESULTS
    from concourse.bass_utils import run_bass_kernel_spmd

    x = np.asarray(x, dtype=np.float32)
    W1 = np.asarray(W1, dtype=np.float32)
    b1 = np.asarray(b1, dtype=np.float32)
    W_mu = np.asarray(W_mu, dtype=np.float32)
    b_mu = np.asarray(b_mu, dtype=np.float32)
    W_logstd = np.asarray(W_logstd, dtype=np.float32)
    b_logstd = np.asarray(b_logstd, dtype=np.float32)

    key = np.asarray(edge_index).tobytes()[:64] + np.asarray(edge_index).tobytes()[-64:]
    cached = _CACHE.get("k")
    if cached is not None and cached[0] == key:
        _, dis, meta, nc = cached
    else:
        dis, meta = _preprocess(edge_index)
        nc = _build_nc(meta)
        _CACHE["k"] = (key, dis, meta, nc)

    x2R = np.zeros((NPAD, 128), np.float16)
    x2R[:N] = (x * dis[:, None]).astype(np.float16)
    W1h = W1.astype(np.float16)
    Wch = np.concatenate([W_mu, W_logstd], axis=1).astype(np.float16)
    b1r = np.tile(b1[None, :], (128, 1)).astype(np.float32)
    bcr = np.tile(np.concatenate([b_mu, b_logstd])[None, :], (128, 1)).astype(np.float32)
    disP = np.zeros(NPAD, np.float32)
    disP[:N] = dis
    ident = np.zeros((128, 128), np.uint8)
    ident[np.arange(128), np.arange(128)] = 0x38
    ident = ident.view(ml_dtypes.float8_e4m3)

    in_maps = []
    for c in range(NCORES):
        s1, oh1 = meta["p1"]["per_core"][c]
        s2, oh2 = meta["p2"]["per_core"][c]
        disSh = disP[c * SH:(c + 1) * SH].reshape(NT, 128).T  # [128, NT]
        in_maps.append({
            "x2Rd": x2R,
            "xlocd": np.ascontiguousarray(x2R[c * SH:(c + 1) * SH]),
            "W1d": W1h, "Wcd": Wch, "b1rd": b1r, "bcrd": bcr,
            "disT32d": np.ascontiguousarray(disSh.astype(np.float32)),
            "identd": ident,
            "idx1g0": s1[0], "idx1g1": s1[1],
            "idx2g0": s2[0], "idx2g1": s2[1],
            "oh1d": oh1, "oh2d": oh2,
        })

    res = run_bass_kernel_spmd(nc, in_maps, core_ids=list(range(NCORES)),
                               trace=TRACE)
    LAST_RESULTS = res
    full = np.concatenate([res.results[c]["out_ml"] for c in range(NCORES)],
                          axis=0)[:N]
    mu = np.ascontiguousarray(full[:, :OUT])
    logstd = np.ascontiguousarray(full[:, OUT:])
    return (mu, logstd)


# revision 19
# speedup vs baseline: 2.6979x; 1.0040x over previous
"""GCN encoder (3x GCNConv sharing one normalized adjacency) on 8 TRN2 NeuronCores.

v3 design:
  - Fold sym-norm into per-node scales: pre-scale rows by dis, post-scale
    aggregates by dis[dst].
  - Conv1 gathers directly from a replicated row-major (x*dis) table in HBM
    and aggregates raw input rows TRANSPOSED (psum[feat,dst] += chunk.T@OH);
    W1 is applied once per dst tile afterwards. No dense pre-GEMM, no first
    AllGather.
  - dma_gather descriptor generation runs on one Q7 core pair per SWDGE
    queue (~7.9ns/desc); gathers rotate over 4 queues so 4 pairs generate
    concurrently. 256B random HBM reads then become the wall (~0.35-0.5
    accesses/ns); deep buffering (TB=2 tile batches, 6 gather bufs) keeps
    the SDMA queues full, and per-block source-sorting improves locality.
  - One-hot scatter matrices precomputed on the host in fp8e4 and streamed
    from HBM (no DVE is_equal).
  - Self loops leave the gather streams; each dst tile adds its local rows
    via one identity matmul.
  - The republish AllGather is split in two chunks (tiles 0-24 / 25-48 of
    each shard) so chunk A overlaps the tail of conv1 and pass-2 gathers on
    table A overlap AllGather B. Pass 2 has its own group split (by chunk
    table), idx streams, and one-hots.
  - mu and logstd share one pass: Wc = [W_mu | W_logstd].
"""

import numpy as np
import ml_dtypes

N = 50000
E = 800000
IN = 128
HID = 128
OUT = 64
NCORES = 8
SH = 6272                 # nodes per core (padded)
NPAD = SH * NCORES        # 50176
NT = SH // 128            # 49 dst tiles per core
LO = 32768                # rows in pass-1 "lo" table (int16 limit)
TSPLIT = 25               # pass-2 chunk A = tiles [0,25), B = [25,49)
RA = TSPLIT * 128         # 3200 rows per shard in chunk A
RB = SH - RA              # 3072 rows per shard in chunk B
NROWA = NCORES * RA       # 25600 (< 32767: int16 ok)
NROWB = NCORES * RB       # 24576
TB = 1                    # dst tiles per gather batch
NQ = 4                    # SWDGE queues
AGA_AT = 33               # issue AllGather-A after consuming this tile (pass 1)
STAG = 8                  # pass-2: issue gB(t) after gA(t+STAG-1)

TRACE = False             # test.py sets this for profiling runs
LAST_RESULTS = None       # test.py reads exec_time_ns from here

_CACHE = {}


def _build_streams(es_tab, t, dl, g, ngrp):
    """Build per-core padded gather streams + fp8 one-hots for one pass.

    es_tab: per-message index into its group's table
    t: dst tile; dl: dst lane; g: group id (0..ngrp-1)
    All arrays are lists per core. Returns dict with C [NT,ngrp], offsets,
    per-core idx streams (per group) and OH fp8 arrays.
    """
    cnts = np.zeros((NCORES, NT, ngrp), np.int64)
    ordered = []
    for c in range(NCORES):
        order = np.lexsort((es_tab[c], g[c], t[c]))  # by tile, grp, src (locality)
        e, tt, dd, gg = es_tab[c][order], t[c][order], dl[c][order], g[c][order]
        key = tt * ngrp + gg
        bc = np.bincount(key, minlength=NT * ngrp)
        cnts[c] = bc.reshape(NT, ngrp)
        ordered.append((e, tt, dd, gg, key))

    C = (cnts.max(axis=0) + 127) // 128            # [NT, ngrp]
    K = C.sum(axis=0).astype(np.int64)             # chunks per group stream
    KT = int(C.sum())
    g_off = np.concatenate([np.zeros((1, ngrp), np.int64),
                            np.cumsum(C, axis=0)[:-1]], axis=0)  # [NT, ngrp]
    kk_off = np.concatenate([[0], np.cumsum(C.sum(axis=1))[:-1]])

    per_core = []
    for c in range(NCORES):
        e, tt, dd, gg, key = ordered[c]
        blk_start = np.concatenate([[0], np.cumsum(cnts[c].reshape(-1))[:-1]])
        rank = np.arange(len(e)) - blk_start[key]
        pos = g_off[tt, gg] * 128 + rank           # position in group stream
        streams = []
        for gi in range(ngrp):
            s = np.zeros(int(K[gi]) * 128, np.int16)
            m = gg == gi
            s[pos[m]] = e[m].astype(np.int16)
            streams.append(np.tile(s.reshape(-1, 16).T, (8, 1)))  # [128, K*8]
        # chunk index: tile-major, groups in order within tile
        cum_in_tile = np.concatenate(
            [np.zeros((NT, 1), np.int64), np.cumsum(C, axis=1)[:, :-1]], axis=1)
        kk = kk_off[tt] + cum_in_tile[tt, gg] + rank // 128
        oh = np.zeros((128, KT * 128), np.uint8)
        oh[rank % 128, kk * 128 + dd] = 0x38       # 1.0 in fp8 e4m3
        per_core.append((streams, oh.view(ml_dtypes.float8_e4m3)))

    return dict(C=C, K=K, KT=KT, g_off=g_off, kk_off=kk_off,
                per_core=per_core, ngrp=ngrp)


def _preprocess(edge_index):
    src = np.asarray(edge_index[0]).astype(np.int64)
    dst = np.asarray(edge_index[1]).astype(np.int64)
    loop = np.arange(N, dtype=np.int64)
    dst_all = np.concatenate([dst, loop])

    deg = np.bincount(dst_all, minlength=N).astype(np.float32)
    dis = (1.0 / np.sqrt(deg)).astype(np.float32)

    es_by_core, t_by_core, dl_by_core = [], [], []
    for c in range(NCORES):
        m = (dst // SH) == c
        es = src[m]
        ed = dst[m] - c * SH
        es_by_core.append(es)
        t_by_core.append(ed >> 7)
        dl_by_core.append(ed & 127)

    # pass 1: table = x2R [NPAD,128]; groups lo (idx<LO) / hi
    g1 = [(e >= LO).astype(np.int64) for e in es_by_core]
    e1 = [np.where(e >= LO, e - LO, e) for e in es_by_core]
    p1 = _build_streams(e1, t_by_core, dl_by_core, g1, 2)

    # pass 2: tables hcfA [NROWA,128] / hcfB [NROWB,128]
    # node (c2, r) -> table A pos c2*RA + r if r < RA else B pos c2*RB + (r-RA)
    g2, e2 = [], []
    for c in range(NCORES):
        e = es_by_core[c]
        c2, r = e // SH, e % SH
        inB = (r >= RA).astype(np.int64)
        g2.append(inB)
        e2.append(np.where(inB == 1, c2 * RB + (r - RA), c2 * RA + r))
    p2 = _build_streams(e2, t_by_core, dl_by_core, g2, 2)

    batches = []
    t0 = 0
    while t0 < NT:
        t1 = min(t0 + TB, NT)
        batches.append((t0, t1))
        t0 = t1
    return dis, dict(p1=p1, p2=p2, batches=batches)


def _build_nc(meta):
    import concourse.bass as bass
    import concourse.bacc as bacc
    import concourse.mybir as mybir
    import concourse.tile as tile
    from concourse import library_config

    batches = meta["batches"]
    p1, p2 = meta["p1"], meta["p2"]

    f16 = mybir.dt.float16
    f32 = mybir.dt.float32
    f8 = mybir.dt.float8e4
    i16 = mybir.dt.int16
    mult = mybir.AluOpType.mult
    add = mybir.AluOpType.add

    nc = bacc.Bacc("TRN2", target_bir_lowering=False, debug=False,
                   enable_asserts=True, num_devices=NCORES,
                   num_swdge_queues=NQ)

    x2Rd = nc.dram_tensor("x2Rd", [NPAD, 128], f16, kind="ExternalInput")
    xlocd = nc.dram_tensor("xlocd", [SH, 128], f16, kind="ExternalInput")
    W1d = nc.dram_tensor("W1d", [128, 128], f16, kind="ExternalInput")
    Wcd = nc.dram_tensor("Wcd", [128, 128], f16, kind="ExternalInput")
    b1rd = nc.dram_tensor("b1rd", [128, 128], f32, kind="ExternalInput")
    bcrd = nc.dram_tensor("bcrd", [128, 128], f32, kind="ExternalInput")
    disT32d = nc.dram_tensor("disT32d", [128, NT], f32, kind="ExternalInput")
    identd = nc.dram_tensor("identd", [128, 128], f8, kind="ExternalInput")
    idx1 = [nc.dram_tensor(f"idx1g{g}", [128, int(p1["K"][g]) * 8], i16,
                           kind="ExternalInput") for g in range(2)]
    idx2 = [nc.dram_tensor(f"idx2g{g}", [128, int(p2["K"][g]) * 8], i16,
                           kind="ExternalInput") for g in range(2)]
    oh1d = nc.dram_tensor("oh1d", [128, p1["KT"] * 128], f8, kind="ExternalInput")
    oh2d = nc.dram_tensor("oh2d", [128, p2["KT"] * 128], f8, kind="ExternalInput")
    out_ml = nc.dram_tensor("out_ml", [SH, 128], f32, kind="ExternalOutput")

    with tile.TileContext(nc) as tc:
        with (
            tc.tile_pool(name="consts", bufs=1) as cpool,
            tc.tile_pool(name="work", bufs=4) as wpool,
            tc.tile_pool(name="oh", bufs=6) as ohpool,
            tc.tile_pool(name="g0", bufs=10) as gpool0,
            tc.tile_pool(name="g1", bufs=10) as gpool1,
            tc.tile_pool(name="psA", bufs=4, space="PSUM") as psA,
            tc.tile_pool(name="psH", bufs=2, space="PSUM") as psH,
            tc.tile_pool(name="dram", bufs=1, space="DRAM") as dpool,
        ):
            nc.gpsimd.load_library(library_config.mlp)

            W1sb = cpool.tile([128, 128], f16, tag="W1sb")
            Wcsb = cpool.tile([128, 128], f16, tag="Wcsb")
            b1sb = cpool.tile([128, 128], f32, tag="b1sb")
            bcsb = cpool.tile([128, 128], f32, tag="bcsb")
            dis32sb = cpool.tile([128, NT], f32, tag="dis32sb")
            identsb = cpool.tile([128, 128], f8, tag="identsb")
            idx1sb = [cpool.tile([128, int(p1["K"][g]) * 8], i16,
                                 tag=f"idx1g{g}", name=f"idx1sb{g}")
                      for g in range(2)]
            idx2sb = [cpool.tile([128, int(p2["K"][g]) * 8], i16,
                                 tag=f"idx2g{g}", name=f"idx2sb{g}")
                      for g in range(2)]

            xres = cpool.tile([128, NT * 128], f16, tag="xres")
            hsres = cpool.tile([128, NT * 128], f16, tag="hsres")

            nc.sync.dma_start(W1sb[:], W1d.ap())
            nc.sync.dma_start(Wcsb[:], Wcd.ap())
            nc.sync.dma_start(b1sb[:], b1rd.ap())
            nc.sync.dma_start(bcsb[:], bcrd.ap())
            nc.sync.dma_start(dis32sb[:], disT32d.ap())
            nc.sync.dma_start(identsb[:], identd.ap())
            for g in range(2):
                nc.sync.dma_start(idx1sb[g][:], idx1[g].ap())
                nc.sync.dma_start(idx2sb[g][:], idx2[g].ap())
            for t in range(NT):
                nc.sync.dma_start(xres[:, t * 128:(t + 1) * 128],
                                  xlocd[t * 128:(t + 1) * 128, :])

            hcsA = dpool.tile([RA, 128], f16, tag="hcsA")
            hcsB = dpool.tile([RB, 128], f16, tag="hcsB")
            hcfA = dpool.tile([NROWA, 128], f16, tag="hcfA", addr_space="Shared")
            hcfB = dpool.tile([NROWB, 128], f16, tag="hcfB", addr_space="Shared")

            def conv_pass(pp, tables, idxsb, ohd_t, loc_res, is_conv1,
                          mid_cb=None, stag=0):
                C, g_off, kk_off = pp["C"], pp["g_off"], pp["kk_off"]
                gts = [{}, {}]
                ohs = {}

                def issue(t, g):
                    cg = int(C[t, g])
                    if cg == 0:
                        gts[g][t] = None
                        return
                    pool = gpool0 if g == 0 else gpool1
                    gt = pool.tile([128, cg, 128], f16, tag=f"gt{g}",
                                   name=f"gt{g}_{t}")
                    o0 = int(g_off[t, g])
                    nc.gpsimd.dma_gather(
                        gt[:], tables[g],
                        idxsb[g][:, o0 * 8:(o0 + cg) * 8],
                        num_idxs=cg * 128, num_idxs_reg=cg * 128,
                        elem_size=128, single_packet=False,
                        queue_num=(t + 2 * g) % NQ,
                    )
                    gts[g][t] = gt

                for step in range(NT + stag):
                    if step < NT:
                        t = step
                        nbk = int(C[t].sum())
                        ohsb = ohpool.tile([128, nbk * 128], f8, tag="ohsb",
                                           name=f"ohsb_{t}")
                        nc.scalar.dma_start(
                            ohsb[:],
                            ohd_t.ap()[:, int(kk_off[t]) * 128:
                                       (int(kk_off[t]) + nbk) * 128])
                        ohs[t] = ohsb
                        issue(t, 0)
                        if stag == 0:
                            issue(t, 1)
                    if stag and step >= stag - 1 and step - (stag - 1) < NT:
                        issue(step - (stag - 1), 1)
                    tc_ = step - stag if stag else step
                    if tc_ < 0 or tc_ >= NT:
                        continue
                    t = tc_
                    nch = int(C[t].sum())
                    ohsb = ohs.pop(t)
                    ps = psA.tile([128, 128], f32, tag="psA")
                    nc.tensor.matmul(ps[:],
                                     loc_res[:, t * 128:(t + 1) * 128],
                                     identsb[:],
                                     start=True, stop=(nch == 0),
                                     skip_group_check=True)
                    k = 0
                    for g in range(2):
                        gt = gts[g].pop(t)
                        for j2 in range(int(C[t, g])):
                            nc.tensor.matmul(
                                ps[:], gt[:, j2, :],
                                ohsb[:, k * 128:(k + 1) * 128],
                                start=False, stop=(k == nch - 1),
                                skip_group_check=True)
                            k += 1

                    aggT = wpool.tile([128, 128], f16, tag="aggT")
                    nc.scalar.copy(aggT[:], ps[:])
                    psh = psH.tile([128, 128], f32, tag="psH")
                    nc.tensor.matmul(psh[:], aggT[:],
                                     W1sb[:] if is_conv1 else Wcsb[:],
                                     start=True, stop=True,
                                     skip_group_check=True)
                    if is_conv1:
                        # h = relu(dis*psh + b1); hs = dis*h
                        hti = wpool.tile([128, 128], f32, tag="hti")
                        nc.vector.scalar_tensor_tensor(
                            hti[:], psh[:], dis32sb[:, t:t + 1], b1sb[:],
                            mult, add)
                        hct = hsres[:, t * 128:(t + 1) * 128]
                        nc.scalar.activation(
                            hct, hti[:], mybir.ActivationFunctionType.Relu,
                            scale=dis32sb[:, t:t + 1])
                        if t < TSPLIT:
                            nc.sync.dma_start(
                                hcsA[t * 128:(t + 1) * 128, :], hct)
                        else:
                            nc.sync.dma_start(
                                hcsB[(t - TSPLIT) * 128:(t - TSPLIT + 1) * 128, :],
                                hct)
                        if mid_cb is not None and t == AGA_AT:
                            mid_cb()
                    else:
                        ot = wpool.tile([128, 128], f32, tag="ot")
                        nc.vector.scalar_tensor_tensor(
                            ot[:], psh[:], dis32sb[:, t:t + 1], bcsb[:],
                            mult, add)
                        nc.sync.dma_start(out_ml.ap()[t * 128:(t + 1) * 128, :],
                                          ot[:])

            def issue_agA():
                nc.gpsimd.collective_compute(
                    "AllGather", mybir.AluOpType.bypass,
                    replica_groups=[list(range(NCORES))],
                    ins=[hcsA.opt()], outs=[hcfA.opt()],
                )

            conv_pass(p1, [x2Rd[0:LO, :], x2Rd[LO:NPAD, :]], idx1sb, oh1d,
                      xres, True, mid_cb=issue_agA)

            nc.gpsimd.collective_compute(
                "AllGather", mybir.AluOpType.bypass,
                replica_groups=[list(range(NCORES))],
                ins=[hcsB.opt()], outs=[hcfB.opt()],
            )

            conv_pass(p2, [hcfA[:], hcfB[:]], idx2sb, oh2d, hsres, False,
                      stag=STAG)

    nc.compile()
    return nc


def kernel(x, edge_index, W1, b1, W_mu, b_mu, W_logstd, b_logstd):
    global LAST_RESULTS
    from concourse.bass_utils import run_bass_kernel_spmd

    x = np.asarray(x, dtype=np.float32)
    W1 = np.asarray(W1, dtype=np.float32)
    b1 = np.asarray(b1, dtype=np.float32)
    W_mu = np.asarray(W_mu, dtype=np.float32)
    b_mu = np.asarray(b_mu, dtype=np.float32)
    W_logstd = np.asarray(W_logstd, dtype=np.float32)
    b_logstd = np.asarray(b_logstd, dtype=np.float32)

    key = np.asarray(edge_index).tobytes()[:64] + np.asarray(edge_index).tobytes()[-64:]
    cached = _CACHE.get("k")
    if cached is not None and cached[0] == key:
        _, dis, meta, nc = cached
    else:
        dis, meta = _preprocess(edge_index)
        nc = _build_nc(meta)
        _CACHE["k"] = (key, dis, meta, nc)

    x2R = np.zeros((NPAD, 128), np.float16)
    x2R[:N] = (x * dis[:, None]).astype(np.float16)
    W1h = W1.astype(np.float16)
    Wch = np.concatenate([W_mu, W_logstd], axis=1).astype(np.float16)
    b1r = np.tile(b1[None, :], (128, 1)).astype(np.float32)
    bcr = np.tile(np.concatenate([b_mu, b_logstd])[None, :], (128, 1)).astype(np.float32)
    disP = np.zeros(NPAD, np.float32)
    disP[:N] = dis
    ident = np.zeros((128, 128), np.uint8)
    ident[np.arange(128), np.arange(128)] = 0x38
    ident = ident.view(ml_dtypes.float8_e4m3)

    in_maps = []
    for c in range(NCORES):
        s1, oh1 = meta["p1"]["per_core"][c]
        s2, oh2 = meta["p2"]["per_core"][c]
        disSh = disP[c * SH:(c + 1) * SH].reshape(NT, 128).T  # [128, NT]
        in_maps.append({
            "x2Rd": x2R,
            "xlocd": np.ascontiguousarray(x2R[c * SH:(c + 1) * SH]),
            "W1d": W1h, "Wcd": Wch, "b1rd": b1r, "bcrd": bcr,
            "disT32d": np.ascontiguousarray(disSh.astype(np.float32)),
            "identd": ident,
            "idx1g0": s1[0], "idx1g1": s1[1],
            "idx2g0": s2[0], "idx2g1": s2[1],
            "oh1d": oh1, "oh2d": oh2,
        })

    res = run_bass_kernel_spmd(nc, in_maps, core_ids=list(range(NCORES)),
                               trace=TRACE)
    LAST_RESULTS = res
    full = np.concatenate([res.results[c]["out_ml"] for c in range(NCORES)],
                          axis=0)[:N]
    mu = np.ascontiguousarray(full[:, :OUT])
    logstd = np.ascontiguousarray(full[:, OUT:])
    return (mu, logstd)


# revision 35
# speedup vs baseline: 2.7917x; 1.0348x over previous
"""GCN encoder (3x GCNConv sharing one normalized adjacency) on 8 TRN2 NeuronCores.

v3 design:
  - Fold sym-norm into per-node scales: pre-scale rows by dis, post-scale
    aggregates by dis[dst].
  - Conv1 gathers directly from a replicated row-major (x*dis) table in HBM
    and aggregates raw input rows TRANSPOSED (psum[feat,dst] += chunk.T@OH);
    W1 is applied once per dst tile afterwards. No dense pre-GEMM, no first
    AllGather.
  - dma_gather descriptor generation runs on one Q7 core pair per SWDGE
    queue (~7.9ns/desc); gathers rotate over 4 queues so 4 pairs generate
    concurrently. 256B random HBM reads then become the wall (~0.35-0.5
    accesses/ns); deep buffering (TB=2 tile batches, 6 gather bufs) keeps
    the SDMA queues full, and per-block source-sorting improves locality.
  - One-hot scatter matrices precomputed on the host in fp8e4 and streamed
    from HBM (no DVE is_equal).
  - Self loops leave the gather streams; each dst tile adds its local rows
    via one identity matmul.
  - The republish AllGather is split in two chunks (tiles 0-24 / 25-48 of
    each shard) so chunk A overlaps the tail of conv1 and pass-2 gathers on
    table A overlap AllGather B. Pass 2 has its own group split (by chunk
    table), idx streams, and one-hots.
  - mu and logstd share one pass: Wc = [W_mu | W_logstd].
"""

import numpy as np
import ml_dtypes

N = 50000
E = 800000
IN = 128
HID = 128
OUT = 64
NCORES = 8
SH = 6272                 # nodes per core (padded)
NPAD = SH * NCORES        # 50176
NT = SH // 128            # 49 dst tiles per core
LO = 32768                # rows in pass-1 "lo" table (int16 limit)
TSPLIT = 25               # pass-2 chunk A = tiles [0,25), B = [25,49)
RA = TSPLIT * 128         # 3200 rows per shard in chunk A
RB = SH - RA              # 3072 rows per shard in chunk B
NROWA = NCORES * RA       # 25600 (< 32767: int16 ok)
NROWB = NCORES * RB       # 24576
TB = 1                    # dst tiles per gather batch
NQ = 4                    # SWDGE queues
AGA_AT = 33               # issue AllGather-A after consuming this tile (pass 1)
STAG = 8                  # pass-2: issue gB(t) after gA(t+STAG-1)

TRACE = False             # test.py sets this for profiling runs
LAST_RESULTS = None       # test.py reads exec_time_ns from here

_CACHE = {}


def _build_streams(es_tab, t, dl, g, ngrp):
    """Build per-core padded gather streams + fp8 one-hots for one pass.

    es_tab: per-message index into its group's table
    t: dst tile; dl: dst lane; g: group id (0..ngrp-1)
    All arrays are lists per core. Returns dict with C [NT,ngrp], offsets,
    per-core idx streams (per group) and OH fp8 arrays.
    """
    cnts = np.zeros((NCORES, NT, ngrp), np.int64)
    ordered = []
    for c in range(NCORES):
        order = np.lexsort((es_tab[c], g[c], t[c]))  # by tile, grp, src (locality)
        e, tt, dd, gg = es_tab[c][order], t[c][order], dl[c][order], g[c][order]
        key = tt * ngrp + gg
        bc = np.bincount(key, minlength=NT * ngrp)
        cnts[c] = bc.reshape(NT, ngrp)
        ordered.append((e, tt, dd, gg, key))

    C = (cnts.max(axis=0) + 127) // 128            # [NT, ngrp]
    K = C.sum(axis=0).astype(np.int64)             # chunks per group stream
    KT = int(C.sum())
    g_off = np.concatenate([np.zeros((1, ngrp), np.int64),
                            np.cumsum(C, axis=0)[:-1]], axis=0)  # [NT, ngrp]
    kk_off = np.concatenate([[0], np.cumsum(C.sum(axis=1))[:-1]])

    per_core = []
    for c in range(NCORES):
        e, tt, dd, gg, key = ordered[c]
        blk_start = np.concatenate([[0], np.cumsum(cnts[c].reshape(-1))[:-1]])
        rank = np.arange(len(e)) - blk_start[key]
        pos = g_off[tt, gg] * 128 + rank           # position in group stream
        streams = []
        for gi in range(ngrp):
            s = np.zeros(int(K[gi]) * 128, np.int16)
            m = gg == gi
            s[pos[m]] = e[m].astype(np.int16)
            streams.append(np.tile(s.reshape(-1, 16).T, (8, 1)))  # [128, K*8]
        # chunk index: tile-major, groups in order within tile
        cum_in_tile = np.concatenate(
            [np.zeros((NT, 1), np.int64), np.cumsum(C, axis=1)[:, :-1]], axis=1)
        kk = kk_off[tt] + cum_in_tile[tt, gg] + rank // 128
        oh = np.zeros((128, KT * 128), np.uint8)
        oh[rank % 128, kk * 128 + dd] = 0x38       # 1.0 in fp8 e4m3
        per_core.append((streams, oh.view(ml_dtypes.float8_e4m3)))

    return dict(C=C, K=K, KT=KT, g_off=g_off, kk_off=kk_off,
                per_core=per_core, ngrp=ngrp)


def _preprocess(edge_index):
    src = np.asarray(edge_index[0]).astype(np.int64)
    dst = np.asarray(edge_index[1]).astype(np.int64)
    loop = np.arange(N, dtype=np.int64)
    dst_all = np.concatenate([dst, loop])

    deg = np.bincount(dst_all, minlength=N).astype(np.float32)
    dis = (1.0 / np.sqrt(deg)).astype(np.float32)

    es_by_core, t_by_core, dl_by_core = [], [], []
    for c in range(NCORES):
        m = (dst // SH) == c
        es = src[m]
        ed = dst[m] - c * SH
        es_by_core.append(es)
        t_by_core.append(ed >> 7)
        dl_by_core.append(ed & 127)

    # pass 1: table = x2R [NPAD,128]; groups lo (idx<LO) / hi
    g1 = [(e >= LO).astype(np.int64) for e in es_by_core]
    e1 = [np.where(e >= LO, e - LO, e) for e in es_by_core]
    p1 = _build_streams(e1, t_by_core, dl_by_core, g1, 2)

    # pass 2: tables hcfA [NROWA,128] / hcfB [NROWB,128]
    # node (c2, r) -> table A pos c2*RA + r if r < RA else B pos c2*RB + (r-RA)
    g2, e2 = [], []
    for c in range(NCORES):
        e = es_by_core[c]
        c2, r = e // SH, e % SH
        inB = (r >= RA).astype(np.int64)
        g2.append(inB)
        e2.append(np.where(inB == 1, c2 * RB + (r - RA), c2 * RA + r))
    p2 = _build_streams(e2, t_by_core, dl_by_core, g2, 2)

    batches = []
    t0 = 0
    while t0 < NT:
        t1 = min(t0 + TB, NT)
        batches.append((t0, t1))
        t0 = t1
    return dis, dict(p1=p1, p2=p2, batches=batches)


def _build_nc(meta):
    import concourse.bass as bass
    import concourse.bacc as bacc
    import concourse.mybir as mybir
    import concourse.tile as tile
    from concourse import library_config

    batches = meta["batches"]
    p1, p2 = meta["p1"], meta["p2"]

    f16 = mybir.dt.float16
    f32 = mybir.dt.float32
    f8 = mybir.dt.float8e4
    i16 = mybir.dt.int16
    mult = mybir.AluOpType.mult
    add = mybir.AluOpType.add

    nc = bacc.Bacc("TRN2", target_bir_lowering=False, debug=False,
                   enable_asserts=True, num_devices=NCORES,
                   num_swdge_queues=NQ)

    x2Rd = nc.dram_tensor("x2Rd", [NPAD, 128], f16, kind="ExternalInput")
    xlocd = nc.dram_tensor("xlocd", [SH, 128], f16, kind="ExternalInput")
    W1d = nc.dram_tensor("W1d", [128, 128], f16, kind="ExternalInput")
    Wcd = nc.dram_tensor("Wcd", [128, 128], f16, kind="ExternalInput")
    b1rd = nc.dram_tensor("b1rd", [128, 128], f32, kind="ExternalInput")
    bcrd = nc.dram_tensor("bcrd", [128, 128], f32, kind="ExternalInput")
    disT32d = nc.dram_tensor("disT32d", [128, NT], f32, kind="ExternalInput")
    identd = nc.dram_tensor("identd", [128, 128], f8, kind="ExternalInput")
    idx1 = [nc.dram_tensor(f"idx1g{g}", [128, int(p1["K"][g]) * 8], i16,
                           kind="ExternalInput") for g in range(2)]
    idx2 = [nc.dram_tensor(f"idx2g{g}", [128, int(p2["K"][g]) * 8], i16,
                           kind="ExternalInput") for g in range(2)]
    oh1d = nc.dram_tensor("oh1d", [128, p1["KT"] * 128], f8, kind="ExternalInput")
    oh2d = nc.dram_tensor("oh2d", [128, p2["KT"] * 128], f8, kind="ExternalInput")
    out_ml = nc.dram_tensor("out_ml", [SH, 128], f32, kind="ExternalOutput")

    with tile.TileContext(nc) as tc:
        with (
            tc.tile_pool(name="consts", bufs=1) as cpool,
            tc.tile_pool(name="work", bufs=4) as wpool,
            tc.tile_pool(name="oh", bufs=6) as ohpool,
            tc.tile_pool(name="g0", bufs=10) as gpool0,
            tc.tile_pool(name="g1", bufs=10) as gpool1,
            tc.tile_pool(name="psA", bufs=4, space="PSUM") as psA,
            tc.tile_pool(name="psH", bufs=2, space="PSUM") as psH,
            tc.tile_pool(name="dram", bufs=1, space="DRAM") as dpool,
        ):
            nc.gpsimd.load_library(library_config.mlp)

            W1sb = cpool.tile([128, 128], f16, tag="W1sb")
            Wcsb = cpool.tile([128, 128], f16, tag="Wcsb")
            b1sb = cpool.tile([128, 128], f32, tag="b1sb")
            bcsb = cpool.tile([128, 128], f32, tag="bcsb")
            dis32sb = cpool.tile([128, NT], f32, tag="dis32sb")
            identsb = cpool.tile([128, 128], f8, tag="identsb")
            idx1sb = [cpool.tile([128, int(p1["K"][g]) * 8], i16,
                                 tag=f"idx1g{g}", name=f"idx1sb{g}")
                      for g in range(2)]
            idx2sb = [cpool.tile([128, int(p2["K"][g]) * 8], i16,
                                 tag=f"idx2g{g}", name=f"idx2sb{g}")
                      for g in range(2)]

            xres = cpool.tile([128, NT * 128], f16, tag="xres")
            hsres = cpool.tile([128, NT * 128], f16, tag="hsres")

            nc.sync.dma_start(W1sb[:], W1d.ap())
            nc.sync.dma_start(Wcsb[:], Wcd.ap())
            nc.sync.dma_start(b1sb[:], b1rd.ap())
            nc.sync.dma_start(bcsb[:], bcrd.ap())
            nc.sync.dma_start(dis32sb[:], disT32d.ap())
            nc.sync.dma_start(identsb[:], identd.ap())
            for g in range(2):
                nc.sync.dma_start(idx1sb[g][:], idx1[g].ap())
                nc.sync.dma_start(idx2sb[g][:], idx2[g].ap())
            for t in range(NT):
                nc.sync.dma_start(xres[:, t * 128:(t + 1) * 128],
                                  xlocd[t * 128:(t + 1) * 128, :])

            hcsA = dpool.tile([RA, 128], f16, tag="hcsA")
            hcsB = dpool.tile([RB, 128], f16, tag="hcsB")
            hcfA = dpool.tile([NROWA, 128], f16, tag="hcfA", addr_space="Shared")
            hcfB = dpool.tile([NROWB, 128], f16, tag="hcfB", addr_space="Shared")

            def conv_pass(pp, tables, idxsb, ohd_t, loc_res, is_conv1,
                          mid_cb=None, stag=0):
                C, g_off, kk_off = pp["C"], pp["g_off"], pp["kk_off"]
                gts = [{}, {}]
                ohs = {}

                def issue(t, g):
                    cg = int(C[t, g])
                    if cg == 0:
                        gts[g][t] = None
                        return
                    pool = gpool0 if g == 0 else gpool1
                    gt = pool.tile([128, cg, 128], f16, tag=f"gt{g}",
                                   name=f"gt{g}_{t}")
                    o0 = int(g_off[t, g])
                    nc.gpsimd.dma_gather(
                        gt[:], tables[g],
                        idxsb[g][:, o0 * 8:(o0 + cg) * 8],
                        num_idxs=cg * 128, num_idxs_reg=cg * 128,
                        elem_size=128, single_packet=False,
                        queue_num=(t + 2 * g) % NQ,
                    )
                    gts[g][t] = gt

                for step in range(NT + stag):
                    if step < NT:
                        t = step
                        nbk = int(C[t].sum())
                        ohsb = ohpool.tile([128, nbk * 128], f8, tag="ohsb",
                                           name=f"ohsb_{t}")
                        nc.scalar.dma_start(
                            ohsb[:],
                            ohd_t.ap()[:, int(kk_off[t]) * 128:
                                       (int(kk_off[t]) + nbk) * 128])
                        ohs[t] = ohsb
                        issue(t, 0)
                        if stag == 0:
                            issue(t, 1)
                    if stag and step >= stag - 1 and step - (stag - 1) < NT:
                        issue(step - (stag - 1), 1)
                    tc_ = step - stag if stag else step
                    if tc_ < 0 or tc_ >= NT:
                        continue
                    t = tc_
                    nch = int(C[t].sum())
                    ohsb = ohs.pop(t)
                    ps = psA.tile([128, 128], f32, tag="psA")
                    nc.tensor.matmul(ps[:],
                                     loc_res[:, t * 128:(t + 1) * 128],
                                     identsb[:],
                                     start=True, stop=(nch == 0),
                                     skip_group_check=True)
                    k = 0
                    for g in range(2):
                        gt = gts[g].pop(t)
                        for j2 in range(int(C[t, g])):
                            nc.tensor.matmul(
                                ps[:], gt[:, j2, :],
                                ohsb[:, k * 128:(k + 1) * 128],
                                start=False, stop=(k == nch - 1),
                                skip_group_check=True)
                            k += 1

                    aggT = wpool.tile([128, 128], f16, tag="aggT")
                    nc.scalar.copy(aggT[:], ps[:])
                    psh = psH.tile([128, 128], f32, tag="psH")
                    nc.tensor.matmul(psh[:], aggT[:],
                                     W1sb[:] if is_conv1 else Wcsb[:],
                                     start=True, stop=True,
                                     skip_group_check=True)
                    if is_conv1:
                        # h = relu(dis*psh + b1); hs = dis*h
                        hti = wpool.tile([128, 128], f32, tag="hti")
                        nc.vector.scalar_tensor_tensor(
                            hti[:], psh[:], dis32sb[:, t:t + 1], b1sb[:],
                            mult, add)
                        hct = hsres[:, t * 128:(t + 1) * 128]
                        nc.scalar.activation(
                            hct, hti[:], mybir.ActivationFunctionType.Relu,
                            scale=dis32sb[:, t:t + 1])
                        if t < TSPLIT:
                            nc.sync.dma_start(
                                hcsA[t * 128:(t + 1) * 128, :], hct)
                        else:
                            nc.sync.dma_start(
                                hcsB[(t - TSPLIT) * 128:(t - TSPLIT + 1) * 128, :],
                                hct)
                        if mid_cb is not None and t == AGA_AT:
                            mid_cb()
                    else:
                        ot = wpool.tile([128, 128], f32, tag="ot")
                        nc.vector.scalar_tensor_tensor(
                            ot[:], psh[:], dis32sb[:, t:t + 1], bcsb[:],
                            mult, add)
                        nc.sync.dma_start(out_ml.ap()[t * 128:(t + 1) * 128, :],
                                          ot[:])

            def issue_agA():
                nc.gpsimd.collective_compute(
                    "AllGather", mybir.AluOpType.bypass,
                    replica_groups=[list(range(NCORES))],
                    ins=[hcsA.opt()], outs=[hcfA.opt()],
                )

            conv_pass(p1, [x2Rd[0:LO, :], x2Rd[LO:NPAD, :]], idx1sb, oh1d,
                      xres, True, mid_cb=issue_agA)

            nc.gpsimd.collective_compute(
                "AllGather", mybir.AluOpType.bypass,
                replica_groups=[list(range(NCORES))],
                ins=[hcsB.opt()], outs=[hcfB.opt()],
            )

            conv_pass(p2, [hcfA[:], hcfB[:]], idx2sb, oh2d, hsres, False,
                      stag=STAG)

    nc.compile()
    return nc


def kernel(x, edge_index, W1, b1, W_mu, b_mu, W_logstd, b_logstd):
    global LAST_RESULTS
    from concourse.bass_utils import run_bass_kernel_spmd

    x = np.asarray(x, dtype=np.float32)
    W1 = np.asarray(W1, dtype=np.float32)
    b1 = np.asarray(b1, dtype=np.float32)
    W_mu = np.asarray(W_mu, dtype=np.float32)
    b_mu = np.asarray(b_mu, dtype=np.float32)
    W_logstd = np.asarray(W_logstd, dtype=np.float32)
    b_logstd = np.asarray(b_logstd, dtype=np.float32)

    key = np.asarray(edge_index).tobytes()[:64] + np.asarray(edge_index).tobytes()[-64:]
    cached = _CACHE.get("k")
    if cached is not None and cached[0] == key:
        _, dis, meta, nc = cached
    else:
        dis, meta = _preprocess(edge_index)
        nc = _build_nc(meta)
        _CACHE["k"] = (key, dis, meta, nc)

    x2R = np.zeros((NPAD, 128), np.float16)
    x2R[:N] = (x * dis[:, None]).astype(np.float16)
    W1h = W1.astype(np.float16)
    Wch = np.concatenate([W_mu, W_logstd], axis=1).astype(np.float16)
    b1r = np.tile(b1[None, :], (128, 1)).astype(np.float32)
    bcr = np.tile(np.concatenate([b_mu, b_logstd])[None, :], (128, 1)).astype(np.float32)
    disP = np.zeros(NPAD, np.float32)
    disP[:N] = dis
    ident = np.zeros((128, 128), np.uint8)
    ident[np.arange(128), np.arange(128)] = 0x38
    ident = ident.view(ml_dtypes.float8_e4m3)

    in_maps = []
    for c in range(NCORES):
        s1, oh1 = meta["p1"]["per_core"][c]
        s2, oh2 = meta["p2"]["per_core"][c]
        disSh = disP[c * SH:(c + 1) * SH].reshape(NT, 128).T  # [128, NT]
        in_maps.append({
            "x2Rd": x2R,
            "xlocd": np.ascontiguousarray(x2R[c * SH:(c + 1) * SH]),
            "W1d": W1h, "Wcd": Wch, "b1rd": b1r, "bcrd": bcr,
            "disT32d": np.ascontiguousarray(disSh.astype(np.float32)),
            "identd": ident,
            "idx1g0": s1[0], "idx1g1": s1[1],
            "idx2g0": s2[0], "idx2g1": s2[1],
            "oh1d": oh1, "oh2d": oh2,
        })

    res = run_bass_kernel_spmd(nc, in_maps, core_ids=list(range(NCORES)),
                               trace=TRACE)
    LAST_RESULTS = res
    full = np.concatenate([res.results[c]["out_ml"] for c in range(NCORES)],
                          axis=0)[:N]
    mu = np.ascontiguousarray(full[:, :OUT])
    logstd = np.ascontiguousarray(full[:, OUT:])
    return (mu, logstd)
